# revision 29
# baseline (speedup 1.0000x reference)
"""Strided (residue-group) attention for Trainium2, SPMD across 8 NeuronCores.

Problem: x[B=2,S=4096,E=1024] -> qkv proj -> per-(batch,head,residue-group)
attention (stride 8 -> 8 groups of n=512 tokens) -> out proj.

Sharding: by (batch, residue-group).  B*stride = 16 group-instances; each of
the 8 cores owns 2 (batch,group) pairs = 1024 tokens and computes their FULL
output rows (it holds all 16 heads for its tokens).  The residue groups are
independent, so there are no cross-device collectives at all; the host
permutes tokens into group-major order on the way in and inverts on the way
out.

v3 design notes (vs the v2 baseline at 264us):
  - bqk loads right after the first weight block (v2 loaded it LAST, so the
    W1 bias-adds stalled 12us and the PE blocked on PSUM recycling).
  - The softmax-denominator path is restructured.  v2 gathered den rows to
    [16,512], ran one DVE reciprocal (3.3us - reciprocal cost scales with
    the FREE dim at ~8 cyc/elem), then broadcast each row across partitions
    with 8 serialized SWDGE DMAs (~1us each + slow data movement): ~12us of
    dead time per group during which the in-order PE blocked behind outproj
    chains.  v3 gathers the den rows TRANSPOSED into [64,128] (4 chunks of
    128 q-tokens per head; free dim 128 -> reciprocal ~1us), then broadcasts
    reciprocal rows into PSUM with tiny K=2 selector matmuls on the PE
    (sel[2,128] lhsT picks the head-half; 4 matmuls x 128 cols per pair,
    ~0.2us) and the normalize multiplies read the PSUM tile directly.
  - g0's bcast+normalize now completes inside W3 (interleaved with the g1
    v-units), so W4's outproj(g0) filler never stalls; g1's runs in the tail
    interleaved with 4 reserved outproj(g0) filler units.
  - v-tile ones-blocks are memset only for g0; the vfp ring (4 bufs, 4
    tiles/group) hands g1 the same buffers with the ones intact.
  - Everything else as v2: ScalarE runs ONLY softmax Exp; weights resident
    in SBUF in DMA-friendly layouts; v-proj bias folded into the out-proj
    bias host-side; score matmuls alternate PE row groups; PV emitted one
    pair behind its scores; fp16 activations, fp16 output (host upcasts).
"""

import os

import numpy as np

B, S, E = 2, 4096, 1024
H, ST = 16, 8
DH = E // H  # 64
N = S // ST  # 512 tokens per residue group
NCORES = 8
GPC = (B * ST) // NCORES  # 2 (batch,group) pairs per core
TOK = GPC * N  # 1024 tokens per core
P = 128
EC = E // P  # 8 contraction chunks of 128
NB = N // P  # 4 token chunks of 128 per group
FB = 2  # feature blocks of 512 in E
SCALE = 1.0 / float(np.sqrt(DH))

_CACHE: dict = {}


def _build_nc():
    import concourse.bass as bass
    import concourse.bacc as bacc
    import concourse.tile as tile
    from concourse import mybir

    F32 = mybir.dt.float32
    FP16 = mybir.dt.float16
    ADD = mybir.AluOpType.add
    EXP = mybir.ActivationFunctionType.Exp

    nc = bacc.Bacc()
    # layouts chosen for long per-partition contiguous runs (big DMA
    # descriptors) and few dma_start jobs (sequencer trigger cost)
    xt = nc.declare_dram_parameter("xt", [GPC, P, EC, N], FP16, isOutput=False)
    wqk = nc.declare_dram_parameter(
        "wqk", [2, 2, P, 4, EC, P], FP16, isOutput=False
    )  # [which, ft-half, p, ft-lo, c, 128]
    wv = nc.declare_dram_parameter("wv", [P, EC, E], FP16, isOutput=False)
    wo = nc.declare_dram_parameter("wo", [P, FB, EC, 512], FP16, isOutput=False)
    bqk = nc.declare_dram_parameter("bqk", [P, 2, EC], F32, isOutput=False)
    bo = nc.declare_dram_parameter("bo", [E], F32, isOutput=False)
    selb = nc.declare_dram_parameter("selb", [DH, DH * DH], FP16, isOutput=False)
    out = nc.declare_dram_parameter("out", [TOK, E], FP16, isOutput=True)

    with nc.allow_low_precision(reason="fp16 softmax-denominator reciprocal"), \
        tile.TileContext(nc) as tc, tc.tile_pool(name="const", bufs=1) as const, \
        tc.tile_pool(name="xtp", bufs=2) as xtp, \
        tc.tile_pool(name="wqkp", bufs=4) as wqkp, \
        tc.tile_pool(name="qkp", bufs=10) as qkp, \
        tc.tile_pool(name="vfp", bufs=4) as vfp, \
        tc.tile_pool(name="expp", bufs=4) as expp, \
        tc.tile_pool(name="osbp", bufs=18) as osbp, \
        tc.tile_pool(name="denp", bufs=2) as denp, \
        tc.tile_pool(name="otp", bufs=16) as otp, \
        tc.tile_pool(name="outp", bufs=4) as outp, \
        tc.tile_pool(name="psmm", bufs=2, space="PSUM") as psmm, \
        tc.tile_pool(name="pssc", bufs=2, space="PSUM") as pssc, \
        tc.tile_pool(name="psop", bufs=2, space="PSUM") as psop:

        # ---- resident weights / constants ------------------------------
        # Few big DMA jobs on the sync HWDGE queue; jobs complete in issue
        # order at ~370GB/s aggregate, so order = first-need order.  bqk
        # goes second: the W1 bias-adds are what recycle the qk PSUM tiles.
        wqk_half = {}  # (which, fthalf) -> [128, 4, EC, 128]

        def load_wqk(which, fh):
            t = wqkp.tile(
                [P, 4, EC, P], FP16, tag="wqk", name=f"w{which}_{fh}"
            )
            nc.sync.dma_start(out=t, in_=wqk[which, fh])
            wqk_half[(which, fh)] = t

        # wqk(0,0) is split into per-ft tiles and xt0 into half tiles so
        # the first q chains start as soon as their own slices land
        # (deps are tile-granular: a split job into one tile would still
        # gate every reader on ALL the sub-jobs).
        w00f = []
        for fl in range(4):
            t = wqkp.tile([P, EC, P], FP16, tag="w00f", name=f"w00f{fl}")
            w00f.append(t)
        xt0h = [
            xtp.tile([P, 4, N], FP16, tag="xt0h", name=f"xt0{h}")
            for h in range(2)
        ]
        # weights stream on the sync HWDGE queue; x/bias/consts stream on
        # the scalar HWDGE queue (idle until W2's exps).  Two queues means
        # the first chain's two deps complete independently (~12us) instead
        # of queueing behind each other's jobs and semaphore slots.
        nc.sync.dma_start(out=w00f[0], in_=wqk[0, 0, :, 0])
        nc.scalar.dma_start(out=xt0h[0], in_=xt[0, :, 0:4])
        bqk_sb = const.tile([P, 2, EC], F32)
        nc.scalar.dma_start(out=bqk_sb, in_=bqk[:])
        nc.sync.dma_start(out=w00f[1], in_=wqk[0, 0, :, 1])
        nc.scalar.dma_start(out=xt0h[1], in_=xt[0, :, 4:8])
        nc.sync.dma_start(out=w00f[2], in_=wqk[0, 0, :, 2])
        nc.sync.dma_start(out=w00f[3], in_=wqk[0, 0, :, 3])
        load_wqk(0, 1)
        load_wqk(1, 0)
        load_wqk(1, 1)
        xt1_sb = xtp.tile([P, EC, N], FP16, tag="xt", name="xt1")
        nc.scalar.dma_start(out=xt1_sb, in_=xt[1])
        wv_sb = const.tile([P, EC, E], FP16)
        nc.scalar.dma_start(out=wv_sb, in_=wv[:])
        wo_sb = const.tile([P, FB, EC, 512], FP16)
        nc.scalar.dma_start(out=wo_sb, in_=wo[:])

        def xt_c(g, c):
            # [P, N] access to contraction chunk c of group g's tokens
            if g == 0:
                return xt0h[c // 4][:, c % 4]
            return xt1_sb[:, c]

        bo_bc = const.tile([P, E], F32)
        nc.gpsimd.dma_start(out=bo_bc, in_=bo[:].partition_broadcast(P))

        # selector for the reciprocal-broadcast matmuls.  Slice (4*pr+c) of
        # sel_big is a [64,128] lhsT whose only nonzeros route rec4 row
        # 4*(2pr)+c (even head, chunk c) to output partitions 0:64 and row
        # 4*(2pr+1)+c to partitions 64:128 (host-built constant; first
        # needed in W3).
        sel_big = const.tile([DH, DH * DH], FP16)
        nc.scalar.dma_start(out=sel_big, in_=selb[:])

        def wqk_tile(which, ft):
            if which == 0 and ft < 4:
                return w00f[ft]
            return wqk_half[(which, ft // 4)][:, ft % 4]

        # ---- state -----------------------------------------------------
        qts = {0: {}, 1: {}}
        kts = {0: {}, 1: {}}
        vfl = {0: [], 1: []}  # per tt: [128, 16, 128] (head blk = v|ones)
        exs = {}
        osbs = {0: {}, 1: {}}
        den4 = {}  # per g: [64,128], partition 4*h + chunk
        rec4 = {}
        ots = {0: {}, 1: {}}
        pv_pending = []  # deferred PV emission (software pipeline lag)

        def emit_qk_chain(g, which, ft, alt=True):
            # alt: alternate psmm/psop for 4-deep chain pipelining (only when
            # the attention PV isn't competing for psop)
            use_op = alt and (ft % 2 == 1)
            ps = (psop if use_op else psmm).tile(
                [P, N], F32, tag="po" if use_op else "mm"
            )
            wt = wqk_tile(which, ft)
            for c in range(EC):
                nc.tensor.matmul(
                    ps,
                    lhsT=wt[:, c, :],
                    rhs=xt_c(g, c),
                    start=(c == 0),
                    stop=(c == EC - 1),
                )
            t = qkp.tile([P, N], FP16, tag="qt" if which == 0 else "kt")
            nc.vector.tensor_scalar(
                out=t,
                in0=ps,
                scalar1=bqk_sb[:, which, ft : ft + 1],
                scalar2=None,
                op0=ADD,
            )
            (qts if which == 0 else kts)[g][ft] = t

        def alloc_v_tiles(g):
            for t2 in range(NB):
                # [128 k-tok, 16 heads, 128]: head block = [v_h | ones]
                # (even h) or [ones | v_h] (odd h) so PV yields o rows on
                # one partition half and denominator rows on the other
                vt = vfp.tile([P, H, P], FP16, tag="vf")
                nc.vector.memset(vt[:, 0:H:2, DH:P], 1.0)
                nc.vector.memset(vt[:, 1:H:2, 0:DH], 1.0)
                vfl[g].append(vt)

        def emit_v_unit(g, fb, tt):
            if fb == 0 and tt == 0:
                alloc_v_tiles(g)
            use_op = tt % 2 == 1
            ps = (psop if use_op else psmm).tile(
                [P, 512], F32, tag="po" if use_op else "mm"
            )
            for c in range(EC):
                nc.tensor.matmul(
                    ps,
                    lhsT=xt_c(g, c)[:, tt * P : (tt + 1) * P],
                    rhs=wv_sb[:, c, fb * 512 : (fb + 1) * 512],
                    start=(c == 0),
                    stop=(c == EC - 1),
                )
            # scatter the 8 heads' v into the interleaved layout with two
            # strided copies (even heads -> block cols 0:64, odd -> 64:128)
            vt = vfl[g][tt]
            psv = ps.rearrange("p (j o) -> p j o", j=NB, o=P)
            h0 = fb * EC
            nc.vector.tensor_copy(
                out=vt[:, h0 : h0 + EC : 2, 0:DH], in_=psv[:, :, 0:DH]
            )
            nc.vector.tensor_copy(
                out=vt[:, h0 + 1 : h0 + EC : 2, DH:P], in_=psv[:, :, DH:P]
            )

        def emit_scores_half(g, pr, half):
            he, ho = 2 * pr, 2 * pr + 1
            if half == 0:
                for h in (he, ho):
                    exs[(g, h)] = expp.tile(
                        [P, NB, N], FP16, tag="exp", name=f"ex{g}_{h}"
                    )
            sce = pssc.tile([P, 2, N], F32, tag="sc")
            sco = pssc.tile([P, 2, N], F32, tag="sc")
            # alternate PE row groups (0-63 / 64-127) so the two heads'
            # K=64 matmuls stream concurrently on the array
            for cc in range(2):
                c = 2 * half + cc
                nc.tensor.matmul(
                    sce[:, cc],
                    lhsT=kts[g][pr][0:DH, c * P : (c + 1) * P],
                    rhs=qts[g][pr][0:DH, :],
                    start=True,
                    stop=True,
                )
                nc.tensor.matmul(
                    sco[:, cc],
                    lhsT=kts[g][pr][DH:P, c * P : (c + 1) * P],
                    rhs=qts[g][pr][DH:P, :],
                    start=True,
                    stop=True,
                )
            nc.scalar.activation(
                out=exs[(g, he)][:, 2 * half : 2 * half + 2], in_=sce, func=EXP
            )
            nc.scalar.activation(
                out=exs[(g, ho)][:, 2 * half : 2 * half + 2], in_=sco, func=EXP
            )

        def emit_pv(g, pr):
            if g not in den4:
                den4[g] = denp.tile([DH, P], FP16, tag="den", name=f"den{g}")
            for h in (2 * pr, 2 * pr + 1):
                po = psop.tile([P, N], F32, tag="po")
                ex = exs[(g, h)]
                for c in range(NB):
                    nc.tensor.matmul(
                        po,
                        lhsT=vfl[g][c][:, h, :],
                        rhs=ex[:, c, :],
                        start=(c == 0),
                        stop=(c == NB - 1),
                    )
                osb = osbp.tile([P, N], FP16, tag="osb")
                nc.vector.tensor_copy(out=osb, in_=po)
                osbs[g][h] = osb
                # gather this head's replicated denominator row transposed
                # into den4: src [1,512] -> dst 4 contiguous partitions x
                # 128 (chunk-major), partition index 4*h + chunk
                dr = DH if h % 2 == 0 else 0
                src = osb[dr : dr + 1, :]
                nc.gpsimd.dma_start(
                    out=den4[g][4 * h : 4 * h + 4, :], in_=src
                )

        def flush_pv():
            while pv_pending:
                g, pr = pv_pending.pop(0)
                emit_pv(g, pr)

        def queue_pv(g, pr):
            pv_pending.append((g, pr))

        def emit_recip4(g, half=None):
            # half=0 -> pairs 0-3 (rows 0:32), half=1 -> pairs 4-7
            if g not in rec4:
                rec4[g] = denp.tile([DH, P], FP16, tag="rec", name=f"rec{g}")
            if half is None:
                nc.vector.reciprocal(out=rec4[g], in_=den4[g])
            else:
                s = slice(32 * half, 32 * half + 32)
                nc.vector.reciprocal(out=rec4[g][s, :], in_=den4[g][s, :])

        def emit_bcast_norm_pair(g, pr):
            # broadcast the pair's reciprocal rows across the partition
            # halves with 4 K=2 selector matmuls (chunk c -> cols c*128),
            # then normalize straight out of PSUM
            bc = psop.tile([P, N], F32, tag="po")
            # pairs 0-3 live in rec4 rows 0:32, pairs 4-7 in 32:64; slice
            # both operands to that half (shared base partition 0 or 32)
            s = slice(32 * (pr // 4), 32 * (pr // 4) + 32)
            r4 = rec4[g][s, :]
            for c in range(NB):
                nc.tensor.matmul(
                    bc[:, c * P : (c + 1) * P],
                    lhsT=sel_big[s, (4 * pr + c) * P : (4 * pr + c + 1) * P],
                    rhs=r4,
                    start=True,
                    stop=True,
                )
            he, ho = 2 * pr, 2 * pr + 1
            ot = otp.tile([P, N], FP16, tag="ot")
            nc.vector.tensor_mul(
                out=ot[0:DH, :], in0=osbs[g][he][0:DH, :], in1=bc[0:DH, :]
            )
            nc.vector.tensor_mul(
                out=ot[DH:P, :], in0=osbs[g][ho][DH:P, :], in1=bc[DH:P, :]
            )
            ots[g][pr] = ot

        def emit_outproj_unit(g, u, alt=False):
            fb, tt = u // NB, u % NB
            use_op = alt and (u % 2 == 1)
            ps = (psop if use_op else psmm).tile(
                [P, 512], F32, tag="po" if use_op else "mm"
            )
            for dc in range(EC):
                nc.tensor.matmul(
                    ps,
                    lhsT=ots[g][dc][:, tt * P : (tt + 1) * P],
                    rhs=wo_sb[:, fb, dc, :],
                    start=(dc == 0),
                    stop=(dc == EC - 1),
                )
            ob = outp.tile([P, 512], FP16, tag="ob")
            nc.vector.tensor_add(
                out=ob, in0=ps, in1=bo_bc[:, fb * 512 : (fb + 1) * 512]
            )
            nc.sync.dma_start(
                out=out[
                    g * N + tt * P : g * N + (tt + 1) * P, fb * 512 : (fb + 1) * 512
                ],
                in_=ob,
            )

        # ---- program order ---------------------------------------------
        # W1: qkv(g0).  The first q chain paces with the xt DMA stream.
        for ft in range(EC):
            emit_qk_chain(0, 0, ft)
        for ft in range(EC):
            emit_qk_chain(0, 1, ft)
        for fb in range(FB):
            for tt in range(NB):
                emit_v_unit(0, fb, tt)

        # W2: attn(g0) with qk(g1) as PE filler while ACT runs the exps.
        # PV for pair pr is emitted one pair late (scores->exp latency) and
        # AFTER both qk chains, so both bias-adds precede the PV CASTs in
        # DVE order and the next pr's chain never stalls on its PSUM slot.
        for pr in range(EC):
            emit_scores_half(0, pr, 0)
            emit_scores_half(0, pr, 1)
            emit_qk_chain(1, 0, pr, alt=False)
            emit_qk_chain(1, 1, pr, alt=False)
            flush_pv()
            queue_pv(0, pr)
        flush_pv()

        # W3: v(g1); meanwhile DVE inverts the g0 denominators ([64,128]
        # transposed layout -> ~1us) and the g0 bcast+normalize pairs run
        # interleaved with the fb=1 v-units, so all g0 ot tiles are ready
        # well before W4's outproj filler needs them.  The reciprocal
        # halves are emitted BEFORE the v-tile memsets/scatters they'd
        # otherwise queue behind on the in-order DVE.
        emit_recip4(0, half=0)
        for tt in range(NB):
            emit_v_unit(1, 0, tt)
            if tt == 0:
                emit_recip4(0, half=1)
        for tt in range(NB):
            emit_v_unit(1, 1, tt)
            emit_bcast_norm_pair(0, 2 * tt)
            emit_bcast_norm_pair(0, 2 * tt + 1)

        # W4: attn(g1) with outproj(g0) as PE filler for prs 0-5.  The g1
        # denominator pipeline starts mid-W4: pairs 0-3 invert after PV pr3
        # landed, and their bcast+normalize runs as pr6/pr7 filler, so only
        # pairs 4-7 remain for the tail.
        for pr in range(EC):
            emit_scores_half(1, pr, 0)
            emit_scores_half(1, pr, 1)
            if pr < 6:
                emit_outproj_unit(0, pr)
            flush_pv()
            queue_pv(1, pr)
            if pr == 5:
                emit_recip4(1, half=0)
            elif pr == 6:
                emit_bcast_norm_pair(1, 0)
                emit_bcast_norm_pair(1, 1)
            elif pr == 7:
                emit_bcast_norm_pair(1, 2)
                emit_bcast_norm_pair(1, 3)
        flush_pv()

        # W5: invert the remaining g1 denominators, bcast+normalize
        # interleaved with the two reserved outproj(g0) units, then
        # out-proj g1 (alternating PSUM pools) and store.
        emit_recip4(1, half=1)
        emit_outproj_unit(0, 6)
        emit_bcast_norm_pair(1, 4)
        emit_bcast_norm_pair(1, 5)
        emit_outproj_unit(0, 7)
        emit_bcast_norm_pair(1, 6)
        emit_bcast_norm_pair(1, 7)
        for u in range(EC):
            emit_outproj_unit(1, u, alt=True)
    nc.finalize()
    return nc


def _get_nc():
    if "nc" not in _CACHE:
        _CACHE["nc"] = _build_nc()
    return _CACHE["nc"]


def _make_in_maps(x, Wqkv, bqkv, Wout, bout):
    """Host-side sharding: permute tokens to group-major, pre-transpose x,
    pack weights into DMA-friendly resident layouts."""
    x = np.asarray(x, dtype=np.float32)
    Wqkv = np.asarray(Wqkv, dtype=np.float32)
    bqkv = np.asarray(bqkv, dtype=np.float32)
    Wout = np.asarray(Wout, dtype=np.float32)
    bout = np.asarray(bout, dtype=np.float32)

    # group-major token order: x_perm[b, g*N + i] = x[b, i*ST + g]
    x_perm = x.reshape(B, N, ST, E).transpose(0, 2, 1, 3)  # [B, ST, N, E]

    # wqk[which][fh][p][fl][c][j] = W[c*128+p, (fh*4+fl)*128+j]  (q scaled)
    def tile_qk(w):
        return w.reshape(EC, P, 2, 4, P).transpose(2, 1, 3, 0, 4)

    wqk = np.ascontiguousarray(
        np.stack(
            [tile_qk(Wqkv[:, 0:E] * SCALE), tile_qk(Wqkv[:, E : 2 * E])], axis=0
        ).astype(np.float16)
    )
    # wv[p][c][f] = Wv[c*128+p, f]
    wv = np.ascontiguousarray(
        Wqkv[:, 2 * E : 3 * E].reshape(EC, P, E).transpose(1, 0, 2).astype(np.float16)
    )
    # wo[p][fb][dc][j] = Wout[dc*128+p, fb*512+j]
    wo = np.ascontiguousarray(
        Wout.reshape(EC, P, FB, 512).transpose(1, 2, 0, 3).astype(np.float16)
    )
    # bqk[p][which][ft] = bias[ft*128+p]
    bq = (bqkv[0:E] * SCALE).reshape(EC, P).T
    bk = bqkv[E : 2 * E].reshape(EC, P).T
    bqk = np.ascontiguousarray(np.stack([bq, bk], axis=1).astype(np.float32))
    # v bias folds into the out bias: o'/den = o/den + bv
    bo = np.ascontiguousarray(
        (bqkv[2 * E : 3 * E] @ Wout + bout).astype(np.float32)
    )
    # reciprocal-broadcast selector: slice m=4*pr+c routes den4 row
    # 4*(2pr)+c to partition half 0:64 and row 4*(2pr+1)+c to 64:128
    selb = np.zeros((DH, DH * DH), dtype=np.float16)
    for m in range(32):
        pr, c = divmod(m, 4)
        selb[4 * (2 * pr) + c, m * P : m * P + DH] = 1.0
        selb[4 * (2 * pr + 1) + c, m * P + DH : (m + 1) * P] = 1.0

    in_maps = []
    for core in range(NCORES):
        b = core // (NCORES // B)
        g0 = GPC * (core % (NCORES // B))
        xc = x_perm[b, g0 : g0 + GPC].reshape(TOK, E)  # [1024, E]
        # xt[g][p][c][t] = x[g*N + t, c*128 + p]
        xct = np.ascontiguousarray(
            xc.T.reshape(EC, P, GPC, N).transpose(2, 1, 0, 3).astype(np.float16)
        )
        in_maps.append(
            {
                "xt": xct,
                "wqk": wqk,
                "wv": wv,
                "wo": wo,
                "bqk": bqk,
                "bo": bo,
                "selb": selb,
            }
        )
    return in_maps


def kernel(x, Wqkv, bqkv, Wout, bout):
    from concourse.bass_utils import run_bass_kernel_spmd

    nc = _get_nc()
    in_maps = _make_in_maps(x, Wqkv, bqkv, Wout, bout)
    trace = bool(int(os.environ.get("KERNEL_TRACE", "0")))
    res = run_bass_kernel_spmd(
        nc, in_maps, core_ids=list(range(NCORES)), trace=trace
    )
    _CACHE["last_result"] = res

    # reassemble: core outputs are [1024 tok, E] fp16 in group-major order
    out = np.empty((B, S, E), dtype=np.float32)
    for b in range(B):
        per_b = [
            np.asarray(res.results[b * (NCORES // B) + j]["out"], dtype=np.float32)
            for j in range(NCORES // B)
        ]
        perm = np.concatenate(per_b, axis=0)  # [ST*N, E] group-major
        out[b] = perm.reshape(ST, N, E).transpose(1, 0, 2).reshape(S, E)
    return out


# revision 31
# speedup vs baseline: 1.0283x; 1.0283x over previous
"""Strided (residue-group) attention for Trainium2, SPMD across 8 NeuronCores.

Problem: x[B=2,S=4096,E=1024] -> qkv proj -> per-(batch,head,residue-group)
attention (stride 8 -> 8 groups of n=512 tokens) -> out proj.

Sharding: by (batch, residue-group).  B*stride = 16 group-instances; each of
the 8 cores owns 2 (batch,group) pairs = 1024 tokens and computes their FULL
output rows (it holds all 16 heads for its tokens).  The residue groups are
independent, so there are no cross-device collectives at all; the host
permutes tokens into group-major order on the way in and inverts on the way
out.

v3 design notes (vs the v2 baseline at 264us):
  - bqk loads right after the first weight block (v2 loaded it LAST, so the
    W1 bias-adds stalled 12us and the PE blocked on PSUM recycling).
  - The softmax-denominator path is restructured.  v2 gathered den rows to
    [16,512], ran one DVE reciprocal (3.3us - reciprocal cost scales with
    the FREE dim at ~8 cyc/elem), then broadcast each row across partitions
    with 8 serialized SWDGE DMAs (~1us each + slow data movement): ~12us of
    dead time per group during which the in-order PE blocked behind outproj
    chains.  v3 gathers the den rows TRANSPOSED into [64,128] (4 chunks of
    128 q-tokens per head; free dim 128 -> reciprocal ~1us), then broadcasts
    reciprocal rows into PSUM with tiny K=2 selector matmuls on the PE
    (sel[2,128] lhsT picks the head-half; 4 matmuls x 128 cols per pair,
    ~0.2us) and the normalize multiplies read the PSUM tile directly.
  - g0's bcast+normalize now completes inside W3 (interleaved with the g1
    v-units), so W4's outproj(g0) filler never stalls; g1's runs in the tail
    interleaved with 4 reserved outproj(g0) filler units.
  - v-tile ones-blocks are memset only for g0; the vfp ring (4 bufs, 4
    tiles/group) hands g1 the same buffers with the ones intact.
  - Everything else as v2: ScalarE runs ONLY softmax Exp; weights resident
    in SBUF in DMA-friendly layouts; v-proj bias folded into the out-proj
    bias host-side; score matmuls alternate PE row groups; PV emitted one
    pair behind its scores; fp16 activations, fp16 output (host upcasts).
"""

import os

import numpy as np

B, S, E = 2, 4096, 1024
H, ST = 16, 8
DH = E // H  # 64
N = S // ST  # 512 tokens per residue group
NCORES = 8
GPC = (B * ST) // NCORES  # 2 (batch,group) pairs per core
TOK = GPC * N  # 1024 tokens per core
P = 128
EC = E // P  # 8 contraction chunks of 128
NB = N // P  # 4 token chunks of 128 per group
FB = 2  # feature blocks of 512 in E
SCALE = 1.0 / float(np.sqrt(DH))

_CACHE: dict = {}


def _build_nc():
    import concourse.bass as bass
    import concourse.bacc as bacc
    import concourse.tile as tile
    from concourse import mybir

    F32 = mybir.dt.float32
    FP16 = mybir.dt.float16
    ADD = mybir.AluOpType.add
    EXP = mybir.ActivationFunctionType.Exp

    nc = bacc.Bacc()
    # layouts chosen for long per-partition contiguous runs (big DMA
    # descriptors) and few dma_start jobs (sequencer trigger cost)
    xt = nc.declare_dram_parameter("xt", [GPC, P, EC, N], FP16, isOutput=False)
    wqk = nc.declare_dram_parameter(
        "wqk", [2, 2, P, 4, EC, P], FP16, isOutput=False
    )  # [which, ft-half, p, ft-lo, c, 128]
    wv = nc.declare_dram_parameter("wv", [P, EC, E], FP16, isOutput=False)
    wo = nc.declare_dram_parameter("wo", [P, FB, EC, 512], FP16, isOutput=False)
    bqk = nc.declare_dram_parameter("bqk", [P, 2, EC], F32, isOutput=False)
    bo = nc.declare_dram_parameter("bo", [E], F32, isOutput=False)
    selb = nc.declare_dram_parameter("selb", [DH, DH * DH], FP16, isOutput=False)
    out = nc.declare_dram_parameter("out", [TOK, E], FP16, isOutput=True)

    with nc.allow_low_precision(reason="fp16 softmax-denominator reciprocal"), \
        tile.TileContext(nc) as tc, tc.tile_pool(name="const", bufs=1) as const, \
        tc.tile_pool(name="xtp", bufs=2) as xtp, \
        tc.tile_pool(name="wqkp", bufs=4) as wqkp, \
        tc.tile_pool(name="qkp", bufs=10) as qkp, \
        tc.tile_pool(name="vfp", bufs=4) as vfp, \
        tc.tile_pool(name="expp", bufs=4) as expp, \
        tc.tile_pool(name="osbp", bufs=18) as osbp, \
        tc.tile_pool(name="denp", bufs=2) as denp, \
        tc.tile_pool(name="otp", bufs=16) as otp, \
        tc.tile_pool(name="outp", bufs=4) as outp, \
        tc.tile_pool(name="psmm", bufs=2, space="PSUM") as psmm, \
        tc.tile_pool(name="pssc", bufs=2, space="PSUM") as pssc, \
        tc.tile_pool(name="psop", bufs=2, space="PSUM") as psop:

        # ---- resident weights / constants ------------------------------
        # Few big DMA jobs on the sync HWDGE queue; jobs complete in issue
        # order at ~370GB/s aggregate, so order = first-need order.  bqk
        # goes second: the W1 bias-adds are what recycle the qk PSUM tiles.
        wqk_half = {}  # (which, fthalf) -> [128, 4, EC, 128]

        def load_wqk(which, fh):
            t = wqkp.tile(
                [P, 4, EC, P], FP16, tag="wqk", name=f"w{which}_{fh}"
            )
            nc.sync.dma_start(out=t, in_=wqk[which, fh])
            wqk_half[(which, fh)] = t

        # wqk(0,0) is split into per-ft tiles and xt0 into half tiles so
        # the first q chains start as soon as their own slices land
        # (deps are tile-granular: a split job into one tile would still
        # gate every reader on ALL the sub-jobs).
        w00f = []
        for fl in range(4):
            t = wqkp.tile([P, EC, P], FP16, tag="w00f", name=f"w00f{fl}")
            w00f.append(t)
        xt0h = [
            xtp.tile([P, 4, N], FP16, tag="xt0h", name=f"xt0{h}")
            for h in range(2)
        ]
        # Jobs within one HWDGE queue interleave across the DMA engines, so
        # everything outstanding completes roughly together: the only lever
        # is keeping each queue's early byte-count small.  The tiny critical
        # prefix (xt0 halves + bqk, ~1MB) goes on the otherwise-idle scalar
        # queue; the weight stream and the late consts stay on sync in
        # need order.
        nc.sync.dma_start(out=w00f[0], in_=wqk[0, 0, :, 0])
        nc.scalar.dma_start(out=xt0h[0], in_=xt[0, :, 0:4])
        bqk_sb = const.tile([P, 2, EC], F32)
        nc.scalar.dma_start(out=bqk_sb, in_=bqk[:])
        nc.sync.dma_start(out=w00f[1], in_=wqk[0, 0, :, 1])
        nc.scalar.dma_start(out=xt0h[1], in_=xt[0, :, 4:8])
        nc.sync.dma_start(out=w00f[2], in_=wqk[0, 0, :, 2])
        nc.sync.dma_start(out=w00f[3], in_=wqk[0, 0, :, 3])
        load_wqk(0, 1)
        load_wqk(1, 0)
        load_wqk(1, 1)
        xt1_sb = xtp.tile([P, EC, N], FP16, tag="xt", name="xt1")
        nc.sync.dma_start(out=xt1_sb, in_=xt[1])
        wv_sb = const.tile([P, EC, E], FP16)
        nc.sync.dma_start(out=wv_sb, in_=wv[:])
        wo_sb = const.tile([P, FB, EC, 512], FP16)
        nc.sync.dma_start(out=wo_sb, in_=wo[:])

        def xt_c(g, c):
            # [P, N] access to contraction chunk c of group g's tokens
            if g == 0:
                return xt0h[c // 4][:, c % 4]
            return xt1_sb[:, c]

        bo_bc = const.tile([P, E], F32)
        nc.gpsimd.dma_start(out=bo_bc, in_=bo[:].partition_broadcast(P))

        # selector for the reciprocal-broadcast matmuls.  Slice (4*pr+c) of
        # sel_big is a [64,128] lhsT whose only nonzeros route rec4 row
        # 4*(2pr)+c (even head, chunk c) to output partitions 0:64 and row
        # 4*(2pr+1)+c to partitions 64:128 (host-built constant; first
        # needed in W3).
        sel_big = const.tile([DH, DH * DH], FP16)
        nc.sync.dma_start(out=sel_big, in_=selb[:])

        def wqk_tile(which, ft):
            if which == 0 and ft < 4:
                return w00f[ft]
            return wqk_half[(which, ft // 4)][:, ft % 4]

        # ---- state -----------------------------------------------------
        qts = {0: {}, 1: {}}
        kts = {0: {}, 1: {}}
        vfl = {0: [], 1: []}  # per tt: [128, 16, 128] (head blk = v|ones)
        exs = {}
        osbs = {0: {}, 1: {}}
        den4 = {}  # per g: [64,128], partition 4*h + chunk
        rec4 = {}
        ots = {0: {}, 1: {}}
        pv_pending = []  # deferred PV emission (software pipeline lag)

        def emit_qk_chain(g, which, ft, alt=True):
            # alt: alternate psmm/psop for 4-deep chain pipelining (only when
            # the attention PV isn't competing for psop)
            use_op = alt and (ft % 2 == 1)
            ps = (psop if use_op else psmm).tile(
                [P, N], F32, tag="po" if use_op else "mm"
            )
            wt = wqk_tile(which, ft)
            for c in range(EC):
                nc.tensor.matmul(
                    ps,
                    lhsT=wt[:, c, :],
                    rhs=xt_c(g, c),
                    start=(c == 0),
                    stop=(c == EC - 1),
                )
            t = qkp.tile([P, N], FP16, tag="qt" if which == 0 else "kt")
            nc.vector.tensor_scalar(
                out=t,
                in0=ps,
                scalar1=bqk_sb[:, which, ft : ft + 1],
                scalar2=None,
                op0=ADD,
            )
            (qts if which == 0 else kts)[g][ft] = t

        def alloc_v_tiles(g):
            for t2 in range(NB):
                # [128 k-tok, 16 heads, 128]: head block = [v_h | ones]
                # (even h) or [ones | v_h] (odd h) so PV yields o rows on
                # one partition half and denominator rows on the other
                vt = vfp.tile([P, H, P], FP16, tag="vf")
                nc.vector.memset(vt[:, 0:H:2, DH:P], 1.0)
                nc.vector.memset(vt[:, 1:H:2, 0:DH], 1.0)
                vfl[g].append(vt)

        def emit_v_unit(g, fb, tt):
            if fb == 0 and tt == 0:
                alloc_v_tiles(g)
            use_op = tt % 2 == 1
            ps = (psop if use_op else psmm).tile(
                [P, 512], F32, tag="po" if use_op else "mm"
            )
            for c in range(EC):
                nc.tensor.matmul(
                    ps,
                    lhsT=xt_c(g, c)[:, tt * P : (tt + 1) * P],
                    rhs=wv_sb[:, c, fb * 512 : (fb + 1) * 512],
                    start=(c == 0),
                    stop=(c == EC - 1),
                )
            # scatter the 8 heads' v into the interleaved layout with two
            # strided copies (even heads -> block cols 0:64, odd -> 64:128)
            vt = vfl[g][tt]
            psv = ps.rearrange("p (j o) -> p j o", j=NB, o=P)
            h0 = fb * EC
            nc.vector.tensor_copy(
                out=vt[:, h0 : h0 + EC : 2, 0:DH], in_=psv[:, :, 0:DH]
            )
            nc.vector.tensor_copy(
                out=vt[:, h0 + 1 : h0 + EC : 2, DH:P], in_=psv[:, :, DH:P]
            )

        def emit_scores_half(g, pr, half):
            he, ho = 2 * pr, 2 * pr + 1
            if half == 0:
                for h in (he, ho):
                    exs[(g, h)] = expp.tile(
                        [P, NB, N], FP16, tag="exp", name=f"ex{g}_{h}"
                    )
            sce = pssc.tile([P, 2, N], F32, tag="sc")
            sco = pssc.tile([P, 2, N], F32, tag="sc")
            # alternate PE row groups (0-63 / 64-127) so the two heads'
            # K=64 matmuls stream concurrently on the array
            for cc in range(2):
                c = 2 * half + cc
                nc.tensor.matmul(
                    sce[:, cc],
                    lhsT=kts[g][pr][0:DH, c * P : (c + 1) * P],
                    rhs=qts[g][pr][0:DH, :],
                    start=True,
                    stop=True,
                )
                nc.tensor.matmul(
                    sco[:, cc],
                    lhsT=kts[g][pr][DH:P, c * P : (c + 1) * P],
                    rhs=qts[g][pr][DH:P, :],
                    start=True,
                    stop=True,
                )
            nc.scalar.activation(
                out=exs[(g, he)][:, 2 * half : 2 * half + 2], in_=sce, func=EXP
            )
            nc.scalar.activation(
                out=exs[(g, ho)][:, 2 * half : 2 * half + 2], in_=sco, func=EXP
            )

        def emit_pv(g, pr):
            if g not in den4:
                den4[g] = denp.tile([DH, P], FP16, tag="den", name=f"den{g}")
            for h in (2 * pr, 2 * pr + 1):
                po = psop.tile([P, N], F32, tag="po")
                ex = exs[(g, h)]
                for c in range(NB):
                    nc.tensor.matmul(
                        po,
                        lhsT=vfl[g][c][:, h, :],
                        rhs=ex[:, c, :],
                        start=(c == 0),
                        stop=(c == NB - 1),
                    )
                osb = osbp.tile([P, N], FP16, tag="osb")
                nc.vector.tensor_copy(out=osb, in_=po)
                osbs[g][h] = osb
                # gather this head's replicated denominator row transposed
                # into den4: src [1,512] -> dst 4 contiguous partitions x
                # 128 (chunk-major), partition index 4*h + chunk
                dr = DH if h % 2 == 0 else 0
                src = osb[dr : dr + 1, :]
                nc.gpsimd.dma_start(
                    out=den4[g][4 * h : 4 * h + 4, :], in_=src
                )

        def flush_pv():
            while pv_pending:
                g, pr = pv_pending.pop(0)
                emit_pv(g, pr)

        def queue_pv(g, pr):
            pv_pending.append((g, pr))

        def emit_recip4(g, half=None):
            # half=0 -> pairs 0-3 (rows 0:32), half=1 -> pairs 4-7
            if g not in rec4:
                rec4[g] = denp.tile([DH, P], FP16, tag="rec", name=f"rec{g}")
            if half is None:
                nc.vector.reciprocal(out=rec4[g], in_=den4[g])
            else:
                s = slice(32 * half, 32 * half + 32)
                nc.vector.reciprocal(out=rec4[g][s, :], in_=den4[g][s, :])

        def emit_bcast_norm_pair(g, pr):
            # broadcast the pair's reciprocal rows across the partition
            # halves with 4 K=2 selector matmuls (chunk c -> cols c*128),
            # then normalize straight out of PSUM
            bc = psop.tile([P, N], F32, tag="po")
            # pairs 0-3 live in rec4 rows 0:32, pairs 4-7 in 32:64; slice
            # both operands to that half (shared base partition 0 or 32)
            s = slice(32 * (pr // 4), 32 * (pr // 4) + 32)
            r4 = rec4[g][s, :]
            for c in range(NB):
                nc.tensor.matmul(
                    bc[:, c * P : (c + 1) * P],
                    lhsT=sel_big[s, (4 * pr + c) * P : (4 * pr + c + 1) * P],
                    rhs=r4,
                    start=True,
                    stop=True,
                )
            he, ho = 2 * pr, 2 * pr + 1
            ot = otp.tile([P, N], FP16, tag="ot")
            nc.vector.tensor_mul(
                out=ot[0:DH, :], in0=osbs[g][he][0:DH, :], in1=bc[0:DH, :]
            )
            nc.vector.tensor_mul(
                out=ot[DH:P, :], in0=osbs[g][ho][DH:P, :], in1=bc[DH:P, :]
            )
            ots[g][pr] = ot

        def emit_outproj_unit(g, u, alt=False):
            fb, tt = u // NB, u % NB
            use_op = alt and (u % 2 == 1)
            ps = (psop if use_op else psmm).tile(
                [P, 512], F32, tag="po" if use_op else "mm"
            )
            for dc in range(EC):
                nc.tensor.matmul(
                    ps,
                    lhsT=ots[g][dc][:, tt * P : (tt + 1) * P],
                    rhs=wo_sb[:, fb, dc, :],
                    start=(dc == 0),
                    stop=(dc == EC - 1),
                )
            ob = outp.tile([P, 512], FP16, tag="ob")
            nc.vector.tensor_add(
                out=ob, in0=ps, in1=bo_bc[:, fb * 512 : (fb + 1) * 512]
            )
            nc.sync.dma_start(
                out=out[
                    g * N + tt * P : g * N + (tt + 1) * P, fb * 512 : (fb + 1) * 512
                ],
                in_=ob,
            )

        # ---- program order ---------------------------------------------
        # W1: qkv(g0).  The first q chain paces with the xt DMA stream.
        for ft in range(EC):
            emit_qk_chain(0, 0, ft)
        for ft in range(EC):
            emit_qk_chain(0, 1, ft)
        for fb in range(FB):
            for tt in range(NB):
                emit_v_unit(0, fb, tt)

        # W2: attn(g0) with qk(g1) as PE filler while ACT runs the exps.
        # PV for pair pr is emitted one pair late (scores->exp latency) and
        # AFTER both qk chains, so both bias-adds precede the PV CASTs in
        # DVE order and the next pr's chain never stalls on its PSUM slot.
        for pr in range(EC):
            emit_scores_half(0, pr, 0)
            emit_scores_half(0, pr, 1)
            emit_qk_chain(1, 0, pr, alt=False)
            emit_qk_chain(1, 1, pr, alt=False)
            flush_pv()
            queue_pv(0, pr)
        flush_pv()

        # W3: v(g1); meanwhile DVE inverts the g0 denominators ([64,128]
        # transposed layout -> ~1us) and the g0 bcast+normalize pairs run
        # interleaved with the fb=1 v-units, so all g0 ot tiles are ready
        # well before W4's outproj filler needs them.  The reciprocal
        # halves are emitted BEFORE the v-tile memsets/scatters they'd
        # otherwise queue behind on the in-order DVE.
        emit_recip4(0, half=0)
        for tt in range(NB):
            emit_v_unit(1, 0, tt)
            if tt == 0:
                emit_recip4(0, half=1)
        for tt in range(NB):
            emit_v_unit(1, 1, tt)
            emit_bcast_norm_pair(0, 2 * tt)
            emit_bcast_norm_pair(0, 2 * tt + 1)

        # W4: attn(g1) with outproj(g0) as PE filler for prs 0-5.  The g1
        # denominator pipeline starts mid-W4: pairs 0-3 invert after PV pr3
        # landed, and their bcast+normalize runs as pr6/pr7 filler, so only
        # pairs 4-7 remain for the tail.
        for pr in range(EC):
            emit_scores_half(1, pr, 0)
            emit_scores_half(1, pr, 1)
            if pr < 6:
                emit_outproj_unit(0, pr)
            flush_pv()
            queue_pv(1, pr)
            if pr == 5:
                emit_recip4(1, half=0)
            elif pr == 6:
                emit_bcast_norm_pair(1, 0)
                emit_bcast_norm_pair(1, 1)
            elif pr == 7:
                emit_bcast_norm_pair(1, 2)
                emit_bcast_norm_pair(1, 3)
        flush_pv()

        # W5: invert the remaining g1 denominators, bcast+normalize
        # interleaved with the two reserved outproj(g0) units, then
        # out-proj g1 (alternating PSUM pools) and store.
        emit_recip4(1, half=1)
        emit_outproj_unit(0, 6)
        emit_bcast_norm_pair(1, 4)
        emit_bcast_norm_pair(1, 5)
        emit_outproj_unit(0, 7)
        emit_bcast_norm_pair(1, 6)
        emit_bcast_norm_pair(1, 7)
        for u in range(EC):
            emit_outproj_unit(1, u, alt=True)
    nc.finalize()
    return nc


def _get_nc():
    if "nc" not in _CACHE:
        _CACHE["nc"] = _build_nc()
    return _CACHE["nc"]


def _make_in_maps(x, Wqkv, bqkv, Wout, bout):
    """Host-side sharding: permute tokens to group-major, pre-transpose x,
    pack weights into DMA-friendly resident layouts."""
    x = np.asarray(x, dtype=np.float32)
    Wqkv = np.asarray(Wqkv, dtype=np.float32)
    bqkv = np.asarray(bqkv, dtype=np.float32)
    Wout = np.asarray(Wout, dtype=np.float32)
    bout = np.asarray(bout, dtype=np.float32)

    # group-major token order: x_perm[b, g*N + i] = x[b, i*ST + g]
    x_perm = x.reshape(B, N, ST, E).transpose(0, 2, 1, 3)  # [B, ST, N, E]

    # wqk[which][fh][p][fl][c][j] = W[c*128+p, (fh*4+fl)*128+j]  (q scaled)
    def tile_qk(w):
        return w.reshape(EC, P, 2, 4, P).transpose(2, 1, 3, 0, 4)

    wqk = np.ascontiguousarray(
        np.stack(
            [tile_qk(Wqkv[:, 0:E] * SCALE), tile_qk(Wqkv[:, E : 2 * E])], axis=0
        ).astype(np.float16)
    )
    # wv[p][c][f] = Wv[c*128+p, f]
    wv = np.ascontiguousarray(
        Wqkv[:, 2 * E : 3 * E].reshape(EC, P, E).transpose(1, 0, 2).astype(np.float16)
    )
    # wo[p][fb][dc][j] = Wout[dc*128+p, fb*512+j]
    wo = np.ascontiguousarray(
        Wout.reshape(EC, P, FB, 512).transpose(1, 2, 0, 3).astype(np.float16)
    )
    # bqk[p][which][ft] = bias[ft*128+p]
    bq = (bqkv[0:E] * SCALE).reshape(EC, P).T
    bk = bqkv[E : 2 * E].reshape(EC, P).T
    bqk = np.ascontiguousarray(np.stack([bq, bk], axis=1).astype(np.float32))
    # v bias folds into the out bias: o'/den = o/den + bv
    bo = np.ascontiguousarray(
        (bqkv[2 * E : 3 * E] @ Wout + bout).astype(np.float32)
    )
    # reciprocal-broadcast selector: slice m=4*pr+c routes den4 row
    # 4*(2pr)+c to partition half 0:64 and row 4*(2pr+1)+c to 64:128
    selb = np.zeros((DH, DH * DH), dtype=np.float16)
    for m in range(32):
        pr, c = divmod(m, 4)
        selb[4 * (2 * pr) + c, m * P : m * P + DH] = 1.0
        selb[4 * (2 * pr + 1) + c, m * P + DH : (m + 1) * P] = 1.0

    in_maps = []
    for core in range(NCORES):
        b = core // (NCORES // B)
        g0 = GPC * (core % (NCORES // B))
        xc = x_perm[b, g0 : g0 + GPC].reshape(TOK, E)  # [1024, E]
        # xt[g][p][c][t] = x[g*N + t, c*128 + p]
        xct = np.ascontiguousarray(
            xc.T.reshape(EC, P, GPC, N).transpose(2, 1, 0, 3).astype(np.float16)
        )
        in_maps.append(
            {
                "xt": xct,
                "wqk": wqk,
                "wv": wv,
                "wo": wo,
                "bqk": bqk,
                "bo": bo,
                "selb": selb,
            }
        )
    return in_maps


def kernel(x, Wqkv, bqkv, Wout, bout):
    from concourse.bass_utils import run_bass_kernel_spmd

    nc = _get_nc()
    in_maps = _make_in_maps(x, Wqkv, bqkv, Wout, bout)
    trace = bool(int(os.environ.get("KERNEL_TRACE", "0")))
    res = run_bass_kernel_spmd(
        nc, in_maps, core_ids=list(range(NCORES)), trace=trace
    )
    _CACHE["last_result"] = res

    # reassemble: core outputs are [1024 tok, E] fp16 in group-major order
    out = np.empty((B, S, E), dtype=np.float32)
    for b in range(B):
        per_b = [
            np.asarray(res.results[b * (NCORES // B) + j]["out"], dtype=np.float32)
            for j in range(NCORES // B)
        ]
        perm = np.concatenate(per_b, axis=0)  # [ST*N, E] group-major
        out[b] = perm.reshape(ST, N, E).transpose(1, 0, 2).reshape(S, E)
    return out


# revision 34
# speedup vs baseline: 1.0462x; 1.0174x over previous
"""Strided (residue-group) attention for Trainium2, SPMD across 8 NeuronCores.

Problem: x[B=2,S=4096,E=1024] -> qkv proj -> per-(batch,head,residue-group)
attention (stride 8 -> 8 groups of n=512 tokens) -> out proj.

Sharding: by (batch, residue-group).  B*stride = 16 group-instances; each of
the 8 cores owns 2 (batch,group) pairs = 1024 tokens and computes their FULL
output rows (it holds all 16 heads for its tokens).  The residue groups are
independent, so there are no cross-device collectives at all; the host
permutes tokens into group-major order on the way in and inverts on the way
out.

v3 design notes (vs the v2 baseline at 264us):
  - bqk loads right after the first weight block (v2 loaded it LAST, so the
    W1 bias-adds stalled 12us and the PE blocked on PSUM recycling).
  - The softmax-denominator path is restructured.  v2 gathered den rows to
    [16,512], ran one DVE reciprocal (3.3us - reciprocal cost scales with
    the FREE dim at ~8 cyc/elem), then broadcast each row across partitions
    with 8 serialized SWDGE DMAs (~1us each + slow data movement): ~12us of
    dead time per group during which the in-order PE blocked behind outproj
    chains.  v3 gathers the den rows TRANSPOSED into [64,128] (4 chunks of
    128 q-tokens per head; free dim 128 -> reciprocal ~1us), then broadcasts
    reciprocal rows into PSUM with tiny K=2 selector matmuls on the PE
    (sel[2,128] lhsT picks the head-half; 4 matmuls x 128 cols per pair,
    ~0.2us) and the normalize multiplies read the PSUM tile directly.
  - g0's bcast+normalize now completes inside W3 (interleaved with the g1
    v-units), so W4's outproj(g0) filler never stalls; g1's runs in the tail
    interleaved with 4 reserved outproj(g0) filler units.
  - v-tile ones-blocks are memset only for g0; the vfp ring (4 bufs, 4
    tiles/group) hands g1 the same buffers with the ones intact.
  - Everything else as v2: ScalarE runs ONLY softmax Exp; weights resident
    in SBUF in DMA-friendly layouts; v-proj bias folded into the out-proj
    bias host-side; score matmuls alternate PE row groups; PV emitted one
    pair behind its scores; fp16 activations, fp16 output (host upcasts).
"""

import os

import numpy as np

B, S, E = 2, 4096, 1024
H, ST = 16, 8
DH = E // H  # 64
N = S // ST  # 512 tokens per residue group
NCORES = 8
GPC = (B * ST) // NCORES  # 2 (batch,group) pairs per core
TOK = GPC * N  # 1024 tokens per core
P = 128
EC = E // P  # 8 contraction chunks of 128
NB = N // P  # 4 token chunks of 128 per group
FB = 2  # feature blocks of 512 in E
SCALE = 1.0 / float(np.sqrt(DH))

_CACHE: dict = {}


def _build_nc():
    import concourse.bass as bass
    import concourse.bacc as bacc
    import concourse.tile as tile
    from concourse import mybir

    F32 = mybir.dt.float32
    FP16 = mybir.dt.float16
    ADD = mybir.AluOpType.add
    EXP = mybir.ActivationFunctionType.Exp

    nc = bacc.Bacc()
    # layouts chosen for long per-partition contiguous runs (big DMA
    # descriptors) and few dma_start jobs (sequencer trigger cost)
    xt = nc.declare_dram_parameter("xt", [GPC, P, EC, N], FP16, isOutput=False)
    wqk = nc.declare_dram_parameter(
        "wqk", [2, 2, P, 4, EC, P], FP16, isOutput=False
    )  # [which, ft-half, p, ft-lo, c, 128]
    wv = nc.declare_dram_parameter("wv", [P, EC, E], FP16, isOutput=False)
    wo = nc.declare_dram_parameter("wo", [P, FB, EC, 512], FP16, isOutput=False)
    bqk = nc.declare_dram_parameter("bqk", [P, 2, EC], F32, isOutput=False)
    bo = nc.declare_dram_parameter("bo", [E], F32, isOutput=False)
    selb = nc.declare_dram_parameter("selb", [DH, DH * DH], FP16, isOutput=False)
    out = nc.declare_dram_parameter("out", [TOK, E], FP16, isOutput=True)

    with nc.allow_low_precision(reason="fp16 softmax-denominator reciprocal"), \
        tile.TileContext(nc) as tc, tc.tile_pool(name="const", bufs=1) as const, \
        tc.tile_pool(name="xtp", bufs=2) as xtp, \
        tc.tile_pool(name="wqkp", bufs=4) as wqkp, \
        tc.tile_pool(name="qkp", bufs=10) as qkp, \
        tc.tile_pool(name="vfp", bufs=4) as vfp, \
        tc.tile_pool(name="expp", bufs=4) as expp, \
        tc.tile_pool(name="osbp", bufs=18) as osbp, \
        tc.tile_pool(name="denp", bufs=2) as denp, \
        tc.tile_pool(name="otp", bufs=16) as otp, \
        tc.tile_pool(name="outp", bufs=4) as outp, \
        tc.tile_pool(name="psmm", bufs=2, space="PSUM") as psmm, \
        tc.tile_pool(name="pssc", bufs=2, space="PSUM") as pssc, \
        tc.tile_pool(name="psop", bufs=2, space="PSUM") as psop:

        # ---- resident weights / constants ------------------------------
        # Few big DMA jobs on the sync HWDGE queue; jobs complete in issue
        # order at ~370GB/s aggregate, so order = first-need order.  bqk
        # goes second: the W1 bias-adds are what recycle the qk PSUM tiles.
        wqk_half = {}  # (which, fthalf) -> [128, 4, EC, 128]

        def load_wqk(which, fh):
            t = wqkp.tile(
                [P, 4, EC, P], FP16, tag="wqk", name=f"w{which}_{fh}"
            )
            nc.sync.dma_start(out=t, in_=wqk[which, fh])
            wqk_half[(which, fh)] = t

        # wqk(0,0) is split into per-ft tiles and xt0 into half tiles so
        # the first q chains start as soon as their own slices land
        # (deps are tile-granular: a split job into one tile would still
        # gate every reader on ALL the sub-jobs).
        w00f = []
        for fl in range(4):
            t = wqkp.tile([P, EC, P], FP16, tag="w00f", name=f"w00f{fl}")
            w00f.append(t)
        xt0h = [
            xtp.tile([P, 4, N], FP16, tag="xt0h", name=f"xt0{h}")
            for h in range(2)
        ]
        # Jobs within one HWDGE queue interleave across the DMA engines, so
        # everything outstanding completes roughly together: the only lever
        # is keeping each queue's early byte-count small.  The tiny critical
        # prefix (xt0 halves + bqk, ~1MB) goes on the otherwise-idle scalar
        # queue; the weight stream and the late consts stay on sync in
        # need order.
        nc.sync.dma_start(out=w00f[0], in_=wqk[0, 0, :, 0])
        nc.sync.dma_start(out=xt0h[0], in_=xt[0, :, 0:4])
        bqk_sb = const.tile([P, 2, EC], F32)
        nc.sync.dma_start(out=bqk_sb, in_=bqk[:])
        nc.sync.dma_start(out=w00f[1], in_=wqk[0, 0, :, 1])
        nc.sync.dma_start(out=xt0h[1], in_=xt[0, :, 4:8])
        nc.sync.dma_start(out=w00f[2], in_=wqk[0, 0, :, 2])
        nc.sync.dma_start(out=w00f[3], in_=wqk[0, 0, :, 3])
        load_wqk(0, 1)
        load_wqk(1, 0)
        load_wqk(1, 1)
        xt1_sb = xtp.tile([P, EC, N], FP16, tag="xt", name="xt1")
        nc.sync.dma_start(out=xt1_sb, in_=xt[1])
        wv_sb = const.tile([P, EC, E], FP16)
        nc.sync.dma_start(out=wv_sb, in_=wv[:])
        wo_sb = const.tile([P, FB, EC, 512], FP16)
        nc.sync.dma_start(out=wo_sb, in_=wo[:])

        def xt_c(g, c):
            # [P, N] access to contraction chunk c of group g's tokens
            if g == 0:
                return xt0h[c // 4][:, c % 4]
            return xt1_sb[:, c]

        bo_bc = const.tile([P, E], F32)
        nc.gpsimd.dma_start(out=bo_bc, in_=bo[:].partition_broadcast(P))

        # selector for the reciprocal-broadcast matmuls.  Slice (4*pr+c) of
        # sel_big is a [64,128] lhsT whose only nonzeros route rec4 row
        # 4*(2pr)+c (even head, chunk c) to output partitions 0:64 and row
        # 4*(2pr+1)+c to partitions 64:128 (host-built constant; first
        # needed in W3).
        sel_big = const.tile([DH, DH * DH], FP16)
        nc.sync.dma_start(out=sel_big, in_=selb[:])

        def wqk_tile(which, ft):
            if which == 0 and ft < 4:
                return w00f[ft]
            return wqk_half[(which, ft // 4)][:, ft % 4]

        # ---- state -----------------------------------------------------
        qts = {0: {}, 1: {}}
        kts = {0: {}, 1: {}}
        vfl = {0: [], 1: []}  # per tt: [128, 16, 128] (head blk = v|ones)
        exs = {}
        osbs = {0: {}, 1: {}}
        den4 = {}  # per g: [64,128], partition 4*h + chunk
        rec4 = {}
        ots = {0: {}, 1: {}}
        pv_pending = []  # deferred PV emission (software pipeline lag)

        def emit_qk_chain(g, which, ft, alt=True):
            # alt: alternate psmm/psop for 4-deep chain pipelining (only when
            # the attention PV isn't competing for psop)
            use_op = alt and (ft % 2 == 1)
            ps = (psop if use_op else psmm).tile(
                [P, N], F32, tag="po" if use_op else "mm"
            )
            wt = wqk_tile(which, ft)
            for c in range(EC):
                nc.tensor.matmul(
                    ps,
                    lhsT=wt[:, c, :],
                    rhs=xt_c(g, c),
                    start=(c == 0),
                    stop=(c == EC - 1),
                )
            t = qkp.tile([P, N], FP16, tag="qt" if which == 0 else "kt")
            nc.vector.tensor_scalar(
                out=t,
                in0=ps,
                scalar1=bqk_sb[:, which, ft : ft + 1],
                scalar2=None,
                op0=ADD,
            )
            (qts if which == 0 else kts)[g][ft] = t

        def alloc_v_tiles(g):
            for t2 in range(NB):
                # [128 k-tok, 16 heads, 128]: head block = [v_h | ones]
                # (even h) or [ones | v_h] (odd h) so PV yields o rows on
                # one partition half and denominator rows on the other
                vt = vfp.tile([P, H, P], FP16, tag="vf")
                nc.vector.memset(vt[:, 0:H:2, DH:P], 1.0)
                nc.vector.memset(vt[:, 1:H:2, 0:DH], 1.0)
                vfl[g].append(vt)

        def emit_v_unit(g, fb, tt):
            if fb == 0 and tt == 0:
                alloc_v_tiles(g)
            use_op = tt % 2 == 1
            ps = (psop if use_op else psmm).tile(
                [P, 512], F32, tag="po" if use_op else "mm"
            )
            for c in range(EC):
                nc.tensor.matmul(
                    ps,
                    lhsT=xt_c(g, c)[:, tt * P : (tt + 1) * P],
                    rhs=wv_sb[:, c, fb * 512 : (fb + 1) * 512],
                    start=(c == 0),
                    stop=(c == EC - 1),
                )
            # scatter the 8 heads' v into the interleaved layout with two
            # strided copies (even heads -> block cols 0:64, odd -> 64:128)
            vt = vfl[g][tt]
            psv = ps.rearrange("p (j o) -> p j o", j=NB, o=P)
            h0 = fb * EC
            nc.vector.tensor_copy(
                out=vt[:, h0 : h0 + EC : 2, 0:DH], in_=psv[:, :, 0:DH]
            )
            nc.vector.tensor_copy(
                out=vt[:, h0 + 1 : h0 + EC : 2, DH:P], in_=psv[:, :, DH:P]
            )

        def emit_scores_half(g, pr, half):
            he, ho = 2 * pr, 2 * pr + 1
            if half == 0:
                for h in (he, ho):
                    exs[(g, h)] = expp.tile(
                        [P, NB, N], FP16, tag="exp", name=f"ex{g}_{h}"
                    )
            sce = pssc.tile([P, 2, N], F32, tag="sc")
            sco = pssc.tile([P, 2, N], F32, tag="sc")
            # alternate PE row groups (0-63 / 64-127) so the two heads'
            # K=64 matmuls stream concurrently on the array
            for cc in range(2):
                c = 2 * half + cc
                nc.tensor.matmul(
                    sce[:, cc],
                    lhsT=kts[g][pr][0:DH, c * P : (c + 1) * P],
                    rhs=qts[g][pr][0:DH, :],
                    start=True,
                    stop=True,
                )
                nc.tensor.matmul(
                    sco[:, cc],
                    lhsT=kts[g][pr][DH:P, c * P : (c + 1) * P],
                    rhs=qts[g][pr][DH:P, :],
                    start=True,
                    stop=True,
                )
            nc.scalar.activation(
                out=exs[(g, he)][:, 2 * half : 2 * half + 2], in_=sce, func=EXP
            )
            nc.scalar.activation(
                out=exs[(g, ho)][:, 2 * half : 2 * half + 2], in_=sco, func=EXP
            )

        def emit_pv(g, pr):
            if g not in den4:
                den4[g] = denp.tile([DH, P], FP16, tag="den", name=f"den{g}")
            for h in (2 * pr, 2 * pr + 1):
                po = psop.tile([P, N], F32, tag="po")
                ex = exs[(g, h)]
                for c in range(NB):
                    nc.tensor.matmul(
                        po,
                        lhsT=vfl[g][c][:, h, :],
                        rhs=ex[:, c, :],
                        start=(c == 0),
                        stop=(c == NB - 1),
                    )
                osb = osbp.tile([P, N], FP16, tag="osb")
                nc.vector.tensor_copy(out=osb, in_=po)
                osbs[g][h] = osb
                # gather this head's replicated denominator row transposed
                # into den4: src [1,512] -> dst 4 contiguous partitions x
                # 128 (chunk-major), partition index 4*h + chunk
                dr = DH if h % 2 == 0 else 0
                src = osb[dr : dr + 1, :]
                nc.gpsimd.dma_start(
                    out=den4[g][4 * h : 4 * h + 4, :], in_=src
                )

        def flush_pv():
            while pv_pending:
                g, pr = pv_pending.pop(0)
                emit_pv(g, pr)

        def queue_pv(g, pr):
            pv_pending.append((g, pr))

        def emit_recip4(g, half=None):
            # half=0 -> pairs 0-3 (rows 0:32), half=1 -> pairs 4-7
            if g not in rec4:
                rec4[g] = denp.tile([DH, P], FP16, tag="rec", name=f"rec{g}")
            if half is None:
                nc.vector.reciprocal(out=rec4[g], in_=den4[g])
            else:
                s = slice(32 * half, 32 * half + 32)
                nc.vector.reciprocal(out=rec4[g][s, :], in_=den4[g][s, :])

        def emit_bcast_norm_pair(g, pr):
            # broadcast the pair's reciprocal rows across the partition
            # halves with 4 K=2 selector matmuls (chunk c -> cols c*128),
            # then normalize straight out of PSUM
            bc = psop.tile([P, N], F32, tag="po")
            # pairs 0-3 live in rec4 rows 0:32, pairs 4-7 in 32:64; slice
            # both operands to that half (shared base partition 0 or 32)
            s = slice(32 * (pr // 4), 32 * (pr // 4) + 32)
            r4 = rec4[g][s, :]
            for c in range(NB):
                nc.tensor.matmul(
                    bc[:, c * P : (c + 1) * P],
                    lhsT=sel_big[s, (4 * pr + c) * P : (4 * pr + c + 1) * P],
                    rhs=r4,
                    start=True,
                    stop=True,
                )
            he, ho = 2 * pr, 2 * pr + 1
            ot = otp.tile([P, N], FP16, tag="ot")
            nc.vector.tensor_mul(
                out=ot[0:DH, :], in0=osbs[g][he][0:DH, :], in1=bc[0:DH, :]
            )
            nc.vector.tensor_mul(
                out=ot[DH:P, :], in0=osbs[g][ho][DH:P, :], in1=bc[DH:P, :]
            )
            ots[g][pr] = ot

        def emit_outproj_unit(g, u, alt=False):
            fb, tt = u // NB, u % NB
            use_op = alt and (u % 2 == 1)
            ps = (psop if use_op else psmm).tile(
                [P, 512], F32, tag="po" if use_op else "mm"
            )
            for dc in range(EC):
                nc.tensor.matmul(
                    ps,
                    lhsT=ots[g][dc][:, tt * P : (tt + 1) * P],
                    rhs=wo_sb[:, fb, dc, :],
                    start=(dc == 0),
                    stop=(dc == EC - 1),
                )
            ob = outp.tile([P, 512], FP16, tag="ob")
            nc.vector.tensor_add(
                out=ob, in0=ps, in1=bo_bc[:, fb * 512 : (fb + 1) * 512]
            )
            nc.sync.dma_start(
                out=out[
                    g * N + tt * P : g * N + (tt + 1) * P, fb * 512 : (fb + 1) * 512
                ],
                in_=ob,
            )

        # ---- program order ---------------------------------------------
        # W1: qkv(g0).  The first q chain paces with the xt DMA stream.
        for ft in range(EC):
            emit_qk_chain(0, 0, ft)
        for ft in range(EC):
            emit_qk_chain(0, 1, ft)
        for fb in range(FB):
            for tt in range(NB):
                emit_v_unit(0, fb, tt)

        # W2: attn(g0) with qk(g1) as PE filler while ACT runs the exps.
        # PV for pair pr is emitted one pair late (scores->exp latency) and
        # AFTER both qk chains, so both bias-adds precede the PV CASTs in
        # DVE order and the next pr's chain never stalls on its PSUM slot.
        # (the ft7 chains are deferred to W4's ACT-bound pr5/pr6 slots)
        for pr in range(EC):
            emit_scores_half(0, pr, 0)
            emit_scores_half(0, pr, 1)
            if pr < 7:
                emit_qk_chain(1, 0, pr, alt=False)
                emit_qk_chain(1, 1, pr, alt=False)
            flush_pv()
            queue_pv(0, pr)
        flush_pv()

        # W3: v(g1); meanwhile DVE inverts the g0 denominators ([64,128]
        # transposed layout -> ~1us) and the g0 bcast+normalize pairs run
        # interleaved with the fb=1 v-units, so all g0 ot tiles are ready
        # well before W4's outproj filler needs them.  The reciprocal
        # halves are emitted BEFORE the v-tile memsets/scatters they'd
        # otherwise queue behind on the in-order DVE.
        emit_recip4(0, half=0)
        for tt in range(NB):
            emit_v_unit(1, 0, tt)
            if tt == 0:
                emit_recip4(0, half=1)
        for tt in range(NB):
            emit_v_unit(1, 1, tt)
            emit_bcast_norm_pair(0, 2 * tt)
            emit_bcast_norm_pair(0, 2 * tt + 1)

        # W4: attn(g1) with outproj(g0) as PE filler for prs 0-5.  The g1
        # denominator pipeline starts mid-W4: pairs 0-3 invert after PV pr3
        # landed, and their bcast+normalize runs as pr6/pr7 filler, so only
        # pairs 4-7 remain for the tail.
        for pr in range(EC):
            emit_scores_half(1, pr, 0)
            emit_scores_half(1, pr, 1)
            if pr < 6:
                emit_outproj_unit(0, pr)
            if pr == 5:
                emit_qk_chain(1, 0, 7, alt=False)
            elif pr == 6:
                emit_qk_chain(1, 1, 7, alt=False)
            flush_pv()
            queue_pv(1, pr)
            if pr == 5:
                emit_recip4(1, half=0)
            elif pr == 6:
                emit_bcast_norm_pair(1, 0)
                emit_bcast_norm_pair(1, 1)
            elif pr == 7:
                emit_bcast_norm_pair(1, 2)
                emit_bcast_norm_pair(1, 3)
        flush_pv()

        # W5: invert the remaining g1 denominators, bcast+normalize
        # interleaved with the two reserved outproj(g0) units, then
        # out-proj g1 (alternating PSUM pools) and store.
        emit_recip4(1, half=1)
        emit_outproj_unit(0, 6)
        emit_bcast_norm_pair(1, 4)
        emit_bcast_norm_pair(1, 5)
        emit_outproj_unit(0, 7)
        emit_bcast_norm_pair(1, 6)
        emit_bcast_norm_pair(1, 7)
        for u in range(EC):
            emit_outproj_unit(1, u, alt=True)
    nc.finalize()
    return nc


def _get_nc():
    if "nc" not in _CACHE:
        _CACHE["nc"] = _build_nc()
    return _CACHE["nc"]


def _make_in_maps(x, Wqkv, bqkv, Wout, bout):
    """Host-side sharding: permute tokens to group-major, pre-transpose x,
    pack weights into DMA-friendly resident layouts."""
    x = np.asarray(x, dtype=np.float32)
    Wqkv = np.asarray(Wqkv, dtype=np.float32)
    bqkv = np.asarray(bqkv, dtype=np.float32)
    Wout = np.asarray(Wout, dtype=np.float32)
    bout = np.asarray(bout, dtype=np.float32)

    # group-major token order: x_perm[b, g*N + i] = x[b, i*ST + g]
    x_perm = x.reshape(B, N, ST, E).transpose(0, 2, 1, 3)  # [B, ST, N, E]

    # wqk[which][fh][p][fl][c][j] = W[c*128+p, (fh*4+fl)*128+j]  (q scaled)
    def tile_qk(w):
        return w.reshape(EC, P, 2, 4, P).transpose(2, 1, 3, 0, 4)

    wqk = np.ascontiguousarray(
        np.stack(
            [tile_qk(Wqkv[:, 0:E] * SCALE), tile_qk(Wqkv[:, E : 2 * E])], axis=0
        ).astype(np.float16)
    )
    # wv[p][c][f] = Wv[c*128+p, f]
    wv = np.ascontiguousarray(
        Wqkv[:, 2 * E : 3 * E].reshape(EC, P, E).transpose(1, 0, 2).astype(np.float16)
    )
    # wo[p][fb][dc][j] = Wout[dc*128+p, fb*512+j]
    wo = np.ascontiguousarray(
        Wout.reshape(EC, P, FB, 512).transpose(1, 2, 0, 3).astype(np.float16)
    )
    # bqk[p][which][ft] = bias[ft*128+p]
    bq = (bqkv[0:E] * SCALE).reshape(EC, P).T
    bk = bqkv[E : 2 * E].reshape(EC, P).T
    bqk = np.ascontiguousarray(np.stack([bq, bk], axis=1).astype(np.float32))
    # v bias folds into the out bias: o'/den = o/den + bv
    bo = np.ascontiguousarray(
        (bqkv[2 * E : 3 * E] @ Wout + bout).astype(np.float32)
    )
    # reciprocal-broadcast selector: slice m=4*pr+c routes den4 row
    # 4*(2pr)+c to partition half 0:64 and row 4*(2pr+1)+c to 64:128
    selb = np.zeros((DH, DH * DH), dtype=np.float16)
    for m in range(32):
        pr, c = divmod(m, 4)
        selb[4 * (2 * pr) + c, m * P : m * P + DH] = 1.0
        selb[4 * (2 * pr + 1) + c, m * P + DH : (m + 1) * P] = 1.0

    in_maps = []
    for core in range(NCORES):
        b = core // (NCORES // B)
        g0 = GPC * (core % (NCORES // B))
        xc = x_perm[b, g0 : g0 + GPC].reshape(TOK, E)  # [1024, E]
        # xt[g][p][c][t] = x[g*N + t, c*128 + p]
        xct = np.ascontiguousarray(
            xc.T.reshape(EC, P, GPC, N).transpose(2, 1, 0, 3).astype(np.float16)
        )
        in_maps.append(
            {
                "xt": xct,
                "wqk": wqk,
                "wv": wv,
                "wo": wo,
                "bqk": bqk,
                "bo": bo,
                "selb": selb,
            }
        )
    return in_maps


def kernel(x, Wqkv, bqkv, Wout, bout):
    from concourse.bass_utils import run_bass_kernel_spmd

    nc = _get_nc()
    in_maps = _make_in_maps(x, Wqkv, bqkv, Wout, bout)
    trace = bool(int(os.environ.get("KERNEL_TRACE", "0")))
    res = run_bass_kernel_spmd(
        nc, in_maps, core_ids=list(range(NCORES)), trace=trace
    )
    _CACHE["last_result"] = res

    # reassemble: core outputs are [1024 tok, E] fp16 in group-major order
    out = np.empty((B, S, E), dtype=np.float32)
    for b in range(B):
        per_b = [
            np.asarray(res.results[b * (NCORES // B) + j]["out"], dtype=np.float32)
            for j in range(NCORES // B)
        ]
        perm = np.concatenate(per_b, axis=0)  # [ST*N, E] group-major
        out[b] = perm.reshape(ST, N, E).transpose(1, 0, 2).reshape(S, E)
    return out


# revision 47
# speedup vs baseline: 1.0467x; 1.0005x over previous
"""Strided (residue-group) attention for Trainium2, SPMD across 8 NeuronCores.

Problem: x[B=2,S=4096,E=1024] -> qkv proj -> per-(batch,head,residue-group)
attention (stride 8 -> 8 groups of n=512 tokens) -> out proj.

Sharding: by (batch, residue-group).  B*stride = 16 group-instances; each of
the 8 cores owns 2 (batch,group) pairs = 1024 tokens and computes their FULL
output rows (it holds all 16 heads for its tokens).  The residue groups are
independent, so there are no cross-device collectives at all; the host
permutes tokens into group-major order on the way in and inverts on the way
out.

v3 design notes (vs the v2 baseline at 264us):
  - bqk loads right after the first weight block (v2 loaded it LAST, so the
    W1 bias-adds stalled 12us and the PE blocked on PSUM recycling).
  - The softmax-denominator path is restructured.  v2 gathered den rows to
    [16,512], ran one DVE reciprocal (3.3us - reciprocal cost scales with
    the FREE dim at ~8 cyc/elem), then broadcast each row across partitions
    with 8 serialized SWDGE DMAs (~1us each + slow data movement): ~12us of
    dead time per group during which the in-order PE blocked behind outproj
    chains.  v3 gathers the den rows TRANSPOSED into [64,128] (4 chunks of
    128 q-tokens per head; free dim 128 -> reciprocal ~1us), then broadcasts
    reciprocal rows into PSUM with tiny K=2 selector matmuls on the PE
    (sel[2,128] lhsT picks the head-half; 4 matmuls x 128 cols per pair,
    ~0.2us) and the normalize multiplies read the PSUM tile directly.
  - g0's bcast+normalize now completes inside W3 (interleaved with the g1
    v-units), so W4's outproj(g0) filler never stalls; g1's runs in the tail
    interleaved with 4 reserved outproj(g0) filler units.
  - v-tile ones-blocks are memset only for g0; the vfp ring (4 bufs, 4
    tiles/group) hands g1 the same buffers with the ones intact.
  - Everything else as v2: ScalarE runs ONLY softmax Exp; weights resident
    in SBUF in DMA-friendly layouts; v-proj bias folded into the out-proj
    bias host-side; score matmuls alternate PE row groups; PV emitted one
    pair behind its scores; fp16 activations, fp16 output (host upcasts).
"""

import os

import numpy as np

B, S, E = 2, 4096, 1024
H, ST = 16, 8
DH = E // H  # 64
N = S // ST  # 512 tokens per residue group
NCORES = 8
GPC = (B * ST) // NCORES  # 2 (batch,group) pairs per core
TOK = GPC * N  # 1024 tokens per core
P = 128
EC = E // P  # 8 contraction chunks of 128
NB = N // P  # 4 token chunks of 128 per group
FB = 2  # feature blocks of 512 in E
SCALE = 1.0 / float(np.sqrt(DH))

_CACHE: dict = {}


def _build_nc():
    import concourse.bass as bass
    import concourse.bacc as bacc
    import concourse.tile as tile
    from concourse import mybir

    F32 = mybir.dt.float32
    FP16 = mybir.dt.float16
    ADD = mybir.AluOpType.add
    EXP = mybir.ActivationFunctionType.Exp

    nc = bacc.Bacc()
    # layouts chosen for long per-partition contiguous runs (big DMA
    # descriptors) and few dma_start jobs (sequencer trigger cost)
    xt = nc.declare_dram_parameter("xt", [GPC, P, EC, N], FP16, isOutput=False)
    wqk = nc.declare_dram_parameter(
        "wqk", [2, 2, P, 4, EC, P], FP16, isOutput=False
    )  # [which, ft-half, p, ft-lo, c, 128]
    wv = nc.declare_dram_parameter("wv", [P, EC, E], FP16, isOutput=False)
    wo = nc.declare_dram_parameter("wo", [P, FB, EC, 512], FP16, isOutput=False)
    bqk = nc.declare_dram_parameter("bqk", [P, 2, EC], F32, isOutput=False)
    bo = nc.declare_dram_parameter("bo", [E], F32, isOutput=False)
    selb = nc.declare_dram_parameter("selb", [DH, DH * DH], FP16, isOutput=False)
    selc = nc.declare_dram_parameter("selc", [16, 8 * P], FP16, isOutput=False)
    out = nc.declare_dram_parameter("out", [TOK, E], FP16, isOutput=True)

    with nc.allow_low_precision(reason="fp16 softmax-denominator reciprocal"), \
        tile.TileContext(nc) as tc, tc.tile_pool(name="const", bufs=1) as const, \
        tc.tile_pool(name="xtp", bufs=2) as xtp, \
        tc.tile_pool(name="wqkp", bufs=4) as wqkp, \
        tc.tile_pool(name="qkp", bufs=10) as qkp, \
        tc.tile_pool(name="vfp", bufs=4) as vfp, \
        tc.tile_pool(name="expp", bufs=4) as expp, \
        tc.tile_pool(name="osbp", bufs=18) as osbp, \
        tc.tile_pool(name="denp", bufs=2) as denp, \
        tc.tile_pool(name="otp", bufs=16) as otp, \
        tc.tile_pool(name="outp", bufs=4) as outp, \
        tc.tile_pool(name="psmm", bufs=2, space="PSUM") as psmm, \
        tc.tile_pool(name="pssc", bufs=2, space="PSUM") as pssc, \
        tc.tile_pool(name="psop", bufs=2, space="PSUM") as psop:

        # ---- resident weights / constants ------------------------------
        # Few big DMA jobs on the sync HWDGE queue; jobs complete in issue
        # order at ~370GB/s aggregate, so order = first-need order.  bqk
        # goes second: the W1 bias-adds are what recycle the qk PSUM tiles.
        wqk_half = {}  # (which, fthalf) -> [128, 4, EC, 128]

        def load_wqk(which, fh):
            t = wqkp.tile(
                [P, 4, EC, P], FP16, tag="wqk", name=f"w{which}_{fh}"
            )
            nc.sync.dma_start(out=t, in_=wqk[which, fh])
            wqk_half[(which, fh)] = t

        # wqk(0,0) is split into per-ft tiles and xt0 into half tiles so
        # the first q chains start as soon as their own slices land
        # (deps are tile-granular: a split job into one tile would still
        # gate every reader on ALL the sub-jobs).
        w00f = []
        for fl in range(4):
            t = wqkp.tile([P, EC, P], FP16, tag="w00f", name=f"w00f{fl}")
            w00f.append(t)
        xt0h = [
            xtp.tile([P, 4, N], FP16, tag="xt0h", name=f"xt0{h}")
            for h in range(2)
        ]
        # Jobs within one HWDGE queue interleave across the DMA engines, so
        # everything outstanding completes roughly together: the only lever
        # is keeping each queue's early byte-count small.  The tiny critical
        # prefix (xt0 halves + bqk, ~1MB) goes on the otherwise-idle scalar
        # queue; the weight stream and the late consts stay on sync in
        # need order.
        nc.sync.dma_start(out=w00f[0], in_=wqk[0, 0, :, 0])
        nc.sync.dma_start(out=xt0h[0], in_=xt[0, :, 0:4])
        bqk_sb = const.tile([P, 2, EC], F32)
        nc.sync.dma_start(out=bqk_sb, in_=bqk[:])
        nc.sync.dma_start(out=w00f[1], in_=wqk[0, 0, :, 1])
        nc.sync.dma_start(out=xt0h[1], in_=xt[0, :, 4:8])
        nc.sync.dma_start(out=w00f[2], in_=wqk[0, 0, :, 2])
        nc.sync.dma_start(out=w00f[3], in_=wqk[0, 0, :, 3])
        load_wqk(0, 1)
        load_wqk(1, 0)
        load_wqk(1, 1)
        xt1_sb = xtp.tile([P, EC, N], FP16, tag="xt", name="xt1")
        nc.sync.dma_start(out=xt1_sb, in_=xt[1])
        wv_sb = const.tile([P, EC, E], FP16)
        nc.sync.dma_start(out=wv_sb, in_=wv[:])
        wo_sb = const.tile([P, FB, EC, 512], FP16)
        nc.sync.dma_start(out=wo_sb, in_=wo[:])

        def xt_c(g, c):
            # [P, N] access to contraction chunk c of group g's tokens
            if g == 0:
                return xt0h[c // 4][:, c % 4]
            return xt1_sb[:, c]

        bo_bc = const.tile([P, E], F32)
        nc.gpsimd.dma_start(out=bo_bc, in_=bo[:].partition_broadcast(P))

        # selector for the reciprocal-broadcast matmuls.  Slice (4*pr+c) of
        # sel_big is a [64,128] lhsT whose only nonzeros route rec4 row
        # 4*(2pr)+c (even head, chunk c) to output partitions 0:64 and row
        # 4*(2pr+1)+c to partitions 64:128 (host-built constant; first
        # needed in W3).
        sel_big = const.tile([DH, DH * DH], FP16)
        nc.sync.dma_start(out=sel_big, in_=selb[:])
        sel_c = const.tile([16, 8 * P], FP16)
        nc.sync.dma_start(out=sel_c, in_=selc[:])

        def wqk_tile(which, ft):
            if which == 0 and ft < 4:
                return w00f[ft]
            return wqk_half[(which, ft // 4)][:, ft % 4]

        # ---- state -----------------------------------------------------
        qts = {0: {}, 1: {}}
        kts = {0: {}, 1: {}}
        vfl = {0: [], 1: []}  # per tt: [128, 16, 128] (head blk = v|ones)
        exs = {}
        osbs = {0: {}, 1: {}}
        den4 = {}  # per g: [48,128] (heads 0-11), partition 4*h + chunk
        den4b = {}  # per g: [16,128] (heads 12-15) at base partition 0,
        rec4 = {}  # so pairs 6-7 can invert right after the last PV lands
        rec4b = {}
        ots = {0: {}, 1: {}}
        pv_pending = []  # deferred PV emission (software pipeline lag)

        def emit_qk_chain(g, which, ft, alt=True):
            # alt: alternate psmm/psop for 4-deep chain pipelining (only when
            # the attention PV isn't competing for psop)
            use_op = alt and (ft % 2 == 1)
            ps = (psop if use_op else psmm).tile(
                [P, N], F32, tag="po" if use_op else "mm"
            )
            wt = wqk_tile(which, ft)
            for c in range(EC):
                nc.tensor.matmul(
                    ps,
                    lhsT=wt[:, c, :],
                    rhs=xt_c(g, c),
                    start=(c == 0),
                    stop=(c == EC - 1),
                )
            t = qkp.tile([P, N], FP16, tag="qt" if which == 0 else "kt")
            nc.vector.tensor_scalar(
                out=t,
                in0=ps,
                scalar1=bqk_sb[:, which, ft : ft + 1],
                scalar2=None,
                op0=ADD,
            )
            (qts if which == 0 else kts)[g][ft] = t

        def alloc_v_tiles(g):
            for t2 in range(NB):
                # [128 k-tok, 16 heads, 128]: head block = [v_h | ones]
                # (even h) or [ones | v_h] (odd h) so PV yields o rows on
                # one partition half and denominator rows on the other
                vt = vfp.tile([P, H, P], FP16, tag="vf")
                nc.vector.memset(vt[:, 0:H:2, DH:P], 1.0)
                nc.vector.memset(vt[:, 1:H:2, 0:DH], 1.0)
                vfl[g].append(vt)

        def emit_v_unit(g, fb, tt):
            if fb == 0 and tt == 0:
                alloc_v_tiles(g)
            use_op = tt % 2 == 1
            ps = (psop if use_op else psmm).tile(
                [P, 512], F32, tag="po" if use_op else "mm"
            )
            for c in range(EC):
                nc.tensor.matmul(
                    ps,
                    lhsT=xt_c(g, c)[:, tt * P : (tt + 1) * P],
                    rhs=wv_sb[:, c, fb * 512 : (fb + 1) * 512],
                    start=(c == 0),
                    stop=(c == EC - 1),
                )
            # scatter the 8 heads' v into the interleaved layout with two
            # strided copies (even heads -> block cols 0:64, odd -> 64:128)
            vt = vfl[g][tt]
            psv = ps.rearrange("p (j o) -> p j o", j=NB, o=P)
            h0 = fb * EC
            nc.vector.tensor_copy(
                out=vt[:, h0 : h0 + EC : 2, 0:DH], in_=psv[:, :, 0:DH]
            )
            nc.vector.tensor_copy(
                out=vt[:, h0 + 1 : h0 + EC : 2, DH:P], in_=psv[:, :, DH:P]
            )

        def emit_scores_half(g, pr, half):
            he, ho = 2 * pr, 2 * pr + 1
            if half == 0:
                for h in (he, ho):
                    exs[(g, h)] = expp.tile(
                        [P, NB, N], FP16, tag="exp", name=f"ex{g}_{h}"
                    )
            sce = pssc.tile([P, 2, N], F32, tag="sc")
            sco = pssc.tile([P, 2, N], F32, tag="sc")
            # alternate PE row groups (0-63 / 64-127) so the two heads'
            # K=64 matmuls stream concurrently on the array
            for cc in range(2):
                c = 2 * half + cc
                nc.tensor.matmul(
                    sce[:, cc],
                    lhsT=kts[g][pr][0:DH, c * P : (c + 1) * P],
                    rhs=qts[g][pr][0:DH, :],
                    start=True,
                    stop=True,
                )
                nc.tensor.matmul(
                    sco[:, cc],
                    lhsT=kts[g][pr][DH:P, c * P : (c + 1) * P],
                    rhs=qts[g][pr][DH:P, :],
                    start=True,
                    stop=True,
                )
            nc.scalar.activation(
                out=exs[(g, he)][:, 2 * half : 2 * half + 2], in_=sce, func=EXP
            )
            nc.scalar.activation(
                out=exs[(g, ho)][:, 2 * half : 2 * half + 2], in_=sco, func=EXP
            )

        def emit_pv(g, pr):
            if g not in den4:
                den4[g] = denp.tile([48, P], FP16, tag="den", name=f"den{g}")
                den4b[g] = denp.tile([16, P], FP16, tag="den", name=f"denb{g}")
            for h in (2 * pr, 2 * pr + 1):
                po = psop.tile([P, N], F32, tag="po")
                ex = exs[(g, h)]
                for c in range(NB):
                    nc.tensor.matmul(
                        po,
                        lhsT=vfl[g][c][:, h, :],
                        rhs=ex[:, c, :],
                        start=(c == 0),
                        stop=(c == NB - 1),
                    )
                osb = osbp.tile([P, N], FP16, tag="osb")
                nc.vector.tensor_copy(out=osb, in_=po)
                osbs[g][h] = osb
                # gather this head's replicated denominator row transposed
                # into den4: src [1,512] -> dst 4 contiguous partitions x
                # 128 (chunk-major), partition index 4*h + chunk
                dr = DH if h % 2 == 0 else 0
                src = osb[dr : dr + 1, :]
                if h < 12:
                    dst = den4[g][4 * h : 4 * h + 4, :]
                else:
                    dst = den4b[g][4 * (h - 12) : 4 * (h - 12) + 4, :]
                nc.gpsimd.dma_start(out=dst, in_=src)

        def flush_pv():
            while pv_pending:
                g, pr = pv_pending.pop(0)
                emit_pv(g, pr)

        def queue_pv(g, pr):
            pv_pending.append((g, pr))

        def emit_recip4(g, part):
            # part 0 -> pairs 0-3 (rows 0:32), 1 -> pairs 4-5 (32:48),
            # 2 -> pairs 6-7 (den4b)
            if g not in rec4:
                rec4[g] = denp.tile([48, P], FP16, tag="rec", name=f"rec{g}")
                rec4b[g] = denp.tile([16, P], FP16, tag="rec", name=f"recb{g}")
            if part == 2:
                nc.vector.reciprocal(out=rec4b[g], in_=den4b[g])
            else:
                s = slice(32 * part, 32 * part + (32 if part == 0 else 16))
                nc.vector.reciprocal(out=rec4[g][s, :], in_=den4[g][s, :])

        def emit_bcast_norm_pair(g, pr):
            # broadcast the pair's reciprocal rows across the partition
            # halves with 4 K=2 selector matmuls (chunk c -> cols c*128),
            # then normalize straight out of PSUM
            bc = psop.tile([P, N], F32, tag="po")
            # operand slices share a legal base partition (0 or 32): pairs
            # 0-3 in rec4[0:32], 4-5 in rec4[32:48], 6-7 in rec4b[0:16]
            if pr < 6:
                s = slice(32 * (pr // 4), 32 * (pr // 4) + (32 if pr < 4 else 16))
                r4 = rec4[g][s, :]
                sel_of = lambda c: sel_big[s, (4 * pr + c) * P : (4 * pr + c + 1) * P]
            else:
                r4 = rec4b[g]
                sel_of = lambda c: sel_c[:, (4 * (pr - 6) + c) * P : (4 * (pr - 6) + c + 1) * P]
            for c in range(NB):
                nc.tensor.matmul(
                    bc[:, c * P : (c + 1) * P],
                    lhsT=sel_of(c),
                    rhs=r4,
                    start=True,
                    stop=True,
                )
            he, ho = 2 * pr, 2 * pr + 1
            ot = otp.tile([P, N], FP16, tag="ot")
            nc.vector.tensor_mul(
                out=ot[0:DH, :], in0=osbs[g][he][0:DH, :], in1=bc[0:DH, :]
            )
            nc.vector.tensor_mul(
                out=ot[DH:P, :], in0=osbs[g][ho][DH:P, :], in1=bc[DH:P, :]
            )
            ots[g][pr] = ot

        def emit_outproj_unit(g, u, alt=False):
            fb, tt = u // NB, u % NB
            use_op = alt and (u % 2 == 1)
            ps = (psop if use_op else psmm).tile(
                [P, 512], F32, tag="po" if use_op else "mm"
            )
            for dc in range(EC):
                nc.tensor.matmul(
                    ps,
                    lhsT=ots[g][dc][:, tt * P : (tt + 1) * P],
                    rhs=wo_sb[:, fb, dc, :],
                    start=(dc == 0),
                    stop=(dc == EC - 1),
                )
            ob = outp.tile([P, 512], FP16, tag="ob")
            nc.vector.tensor_add(
                out=ob, in0=ps, in1=bo_bc[:, fb * 512 : (fb + 1) * 512]
            )
            nc.sync.dma_start(
                out=out[
                    g * N + tt * P : g * N + (tt + 1) * P, fb * 512 : (fb + 1) * 512
                ],
                in_=ob,
            )

        # ---- program order ---------------------------------------------
        # W1: qkv(g0).  The first q chain paces with the xt DMA stream.
        for ft in range(EC):
            emit_qk_chain(0, 0, ft)
        for ft in range(EC):
            emit_qk_chain(0, 1, ft)
        for fb in range(FB):
            for tt in range(NB):
                emit_v_unit(0, fb, tt)

        # W2: attn(g0) with qk(g1) as PE filler while ACT runs the exps.
        # PV for pair pr is emitted one pair late (scores->exp latency) and
        # AFTER both qk chains, so both bias-adds precede the PV CASTs in
        # DVE order and the next pr's chain never stalls on its PSUM slot.
        # (the ft7 chains are deferred to W4's ACT-bound pr5/pr6 slots)
        for pr in range(EC):
            emit_scores_half(0, pr, 0)
            emit_scores_half(0, pr, 1)
            if pr < 7:
                emit_qk_chain(1, 0, pr, alt=False)
                emit_qk_chain(1, 1, pr, alt=False)
            flush_pv()
            queue_pv(0, pr)
        flush_pv()

        # W3: v(g1); meanwhile DVE inverts the g0 denominators ([64,128]
        # transposed layout -> ~1us) and the g0 bcast+normalize pairs run
        # interleaved with the fb=1 v-units, so all g0 ot tiles are ready
        # well before W4's outproj filler needs them.  The reciprocal
        # halves are emitted BEFORE the v-tile memsets/scatters they'd
        # otherwise queue behind on the in-order DVE.
        emit_recip4(0, 0)
        for tt in range(NB):
            emit_v_unit(1, 0, tt)
            if tt == 0:
                emit_recip4(0, 1)
                emit_recip4(0, 2)
        for tt in range(NB):
            emit_v_unit(1, 1, tt)
            emit_bcast_norm_pair(0, 2 * tt)
            emit_bcast_norm_pair(0, 2 * tt + 1)

        # W4: attn(g1) with outproj(g0) as PE filler for prs 0-5.  The g1
        # denominator pipeline starts mid-W4: pairs 0-3 invert after PV pr3
        # landed, and their bcast+normalize runs as pr6/pr7 filler, so only
        # pairs 4-7 remain for the tail.
        for pr in range(EC):
            emit_scores_half(1, pr, 0)
            emit_scores_half(1, pr, 1)
            if pr < 6:
                emit_outproj_unit(0, pr)
            if pr == 5:
                emit_qk_chain(1, 0, 7, alt=False)
            elif pr == 6:
                emit_qk_chain(1, 1, 7, alt=False)
            flush_pv()
            queue_pv(1, pr)
            if pr == 5:
                emit_recip4(1, 0)
            elif pr == 6:
                emit_recip4(1, 1)
                emit_bcast_norm_pair(1, 0)
                emit_bcast_norm_pair(1, 1)
            elif pr == 7:
                emit_bcast_norm_pair(1, 2)
                emit_bcast_norm_pair(1, 3)
                emit_bcast_norm_pair(1, 4)
                emit_bcast_norm_pair(1, 5)
        flush_pv()

        # W5: invert the last g1 denominators (pairs 6-7, base-0 tile so
        # the reciprocal can start right after the last PV), bcast +
        # normalize interleaved with the two reserved outproj(g0) units,
        # then out-proj g1 (alternating PSUM pools) and store.
        emit_recip4(1, 2)
        emit_outproj_unit(0, 6)
        emit_bcast_norm_pair(1, 6)
        emit_outproj_unit(0, 7)
        emit_bcast_norm_pair(1, 7)
        for u in range(EC):
            emit_outproj_unit(1, u, alt=True)
    nc.finalize()
    return nc


def _get_nc():
    if "nc" not in _CACHE:
        _CACHE["nc"] = _build_nc()
    return _CACHE["nc"]


def _make_in_maps(x, Wqkv, bqkv, Wout, bout):
    """Host-side sharding: permute tokens to group-major, pre-transpose x,
    pack weights into DMA-friendly resident layouts."""
    x = np.asarray(x, dtype=np.float32)
    Wqkv = np.asarray(Wqkv, dtype=np.float32)
    bqkv = np.asarray(bqkv, dtype=np.float32)
    Wout = np.asarray(Wout, dtype=np.float32)
    bout = np.asarray(bout, dtype=np.float32)

    # group-major token order: x_perm[b, g*N + i] = x[b, i*ST + g]
    x_perm = x.reshape(B, N, ST, E).transpose(0, 2, 1, 3)  # [B, ST, N, E]

    # wqk[which][fh][p][fl][c][j] = W[c*128+p, (fh*4+fl)*128+j]  (q scaled)
    def tile_qk(w):
        return w.reshape(EC, P, 2, 4, P).transpose(2, 1, 3, 0, 4)

    wqk = np.ascontiguousarray(
        np.stack(
            [tile_qk(Wqkv[:, 0:E] * SCALE), tile_qk(Wqkv[:, E : 2 * E])], axis=0
        ).astype(np.float16)
    )
    # wv[p][c][f] = Wv[c*128+p, f]
    wv = np.ascontiguousarray(
        Wqkv[:, 2 * E : 3 * E].reshape(EC, P, E).transpose(1, 0, 2).astype(np.float16)
    )
    # wo[p][fb][dc][j] = Wout[dc*128+p, fb*512+j]
    wo = np.ascontiguousarray(
        Wout.reshape(EC, P, FB, 512).transpose(1, 2, 0, 3).astype(np.float16)
    )
    # bqk[p][which][ft] = bias[ft*128+p]
    bq = (bqkv[0:E] * SCALE).reshape(EC, P).T
    bk = bqkv[E : 2 * E].reshape(EC, P).T
    bqk = np.ascontiguousarray(np.stack([bq, bk], axis=1).astype(np.float32))
    # v bias folds into the out bias: o'/den = o/den + bv
    bo = np.ascontiguousarray(
        (bqkv[2 * E : 3 * E] @ Wout + bout).astype(np.float32)
    )
    # reciprocal-broadcast selectors: slice m=4*pr+c routes den4 row
    # 4*(2pr)+c to partition half 0:64 and row 4*(2pr+1)+c to 64:128;
    # pairs 6-7 live in their own base-0 tile with selector selc
    selb = np.zeros((DH, DH * DH), dtype=np.float16)
    for m in range(24):
        pr, c = divmod(m, 4)
        selb[4 * (2 * pr) + c, m * P : m * P + DH] = 1.0
        selb[4 * (2 * pr + 1) + c, m * P + DH : (m + 1) * P] = 1.0
    selc = np.zeros((16, 8 * P), dtype=np.float16)
    for m in range(8):
        pr, c = divmod(m, 4)
        selc[4 * (2 * pr) + c, m * P : m * P + DH] = 1.0
        selc[4 * (2 * pr + 1) + c, m * P + DH : (m + 1) * P] = 1.0

    in_maps = []
    for core in range(NCORES):
        b = core // (NCORES // B)
        g0 = GPC * (core % (NCORES // B))
        xc = x_perm[b, g0 : g0 + GPC].reshape(TOK, E)  # [1024, E]
        # xt[g][p][c][t] = x[g*N + t, c*128 + p]
        xct = np.ascontiguousarray(
            xc.T.reshape(EC, P, GPC, N).transpose(2, 1, 0, 3).astype(np.float16)
        )
        in_maps.append(
            {
                "xt": xct,
                "wqk": wqk,
                "wv": wv,
                "wo": wo,
                "bqk": bqk,
                "bo": bo,
                "selb": selb,
                "selc": selc,
            }
        )
    return in_maps


def kernel(x, Wqkv, bqkv, Wout, bout):
    from concourse.bass_utils import run_bass_kernel_spmd

    nc = _get_nc()
    in_maps = _make_in_maps(x, Wqkv, bqkv, Wout, bout)
    trace = bool(int(os.environ.get("KERNEL_TRACE", "0")))
    res = run_bass_kernel_spmd(
        nc, in_maps, core_ids=list(range(NCORES)), trace=trace
    )
    _CACHE["last_result"] = res

    # reassemble: core outputs are [1024 tok, E] fp16 in group-major order
    out = np.empty((B, S, E), dtype=np.float32)
    for b in range(B):
        per_b = [
            np.asarray(res.results[b * (NCORES // B) + j]["out"], dtype=np.float32)
            for j in range(NCORES // B)
        ]
        perm = np.concatenate(per_b, axis=0)  # [ST*N, E] group-major
        out[b] = perm.reshape(ST, N, E).transpose(1, 0, 2).reshape(S, E)
    return out


# revision 60
# speedup vs baseline: 1.0591x; 1.0118x over previous
"""Strided (residue-group) attention for Trainium2, SPMD across 8 NeuronCores.

Problem: x[B=2,S=4096,E=1024] -> qkv proj -> per-(batch,head,residue-group)
attention (stride 8 -> 8 groups of n=512 tokens) -> out proj.

Sharding: by (batch, residue-group).  B*stride = 16 group-instances; each of
the 8 cores owns 2 (batch,group) pairs = 1024 tokens and computes their FULL
output rows (it holds all 16 heads for its tokens).  The residue groups are
independent, so there are no cross-device collectives at all; the host
permutes tokens into group-major order on the way in and inverts on the way
out.

v3 design notes (vs the v2 baseline at 264us):
  - bqk loads right after the first weight block (v2 loaded it LAST, so the
    W1 bias-adds stalled 12us and the PE blocked on PSUM recycling).
  - The softmax-denominator path is restructured.  v2 gathered den rows to
    [16,512], ran one DVE reciprocal (3.3us - reciprocal cost scales with
    the FREE dim at ~8 cyc/elem), then broadcast each row across partitions
    with 8 serialized SWDGE DMAs (~1us each + slow data movement): ~12us of
    dead time per group during which the in-order PE blocked behind outproj
    chains.  v3 gathers the den rows TRANSPOSED into [64,128] (4 chunks of
    128 q-tokens per head; free dim 128 -> reciprocal ~1us), then broadcasts
    reciprocal rows into PSUM with tiny K=2 selector matmuls on the PE
    (sel[2,128] lhsT picks the head-half; 4 matmuls x 128 cols per pair,
    ~0.2us) and the normalize multiplies read the PSUM tile directly.
  - g0's bcast+normalize now completes inside W3 (interleaved with the g1
    v-units), so W4's outproj(g0) filler never stalls; g1's runs in the tail
    interleaved with 4 reserved outproj(g0) filler units.
  - v-tile ones-blocks are memset only for g0; the vfp ring (4 bufs, 4
    tiles/group) hands g1 the same buffers with the ones intact.
  - Everything else as v2: ScalarE runs ONLY softmax Exp; weights resident
    in SBUF in DMA-friendly layouts; v-proj bias folded into the out-proj
    bias host-side; score matmuls alternate PE row groups; PV emitted one
    pair behind its scores; fp16 activations, fp16 output (host upcasts).
"""

import os

import numpy as np

B, S, E = 2, 4096, 1024
H, ST = 16, 8
DH = E // H  # 64
N = S // ST  # 512 tokens per residue group
NCORES = 8
GPC = (B * ST) // NCORES  # 2 (batch,group) pairs per core
TOK = GPC * N  # 1024 tokens per core
P = 128
EC = E // P  # 8 contraction chunks of 128
NB = N // P  # 4 token chunks of 128 per group
FB = 2  # feature blocks of 512 in E
SCALE = 1.0 / float(np.sqrt(DH))

_CACHE: dict = {}


def _build_nc():
    import concourse.bass as bass
    import concourse.bacc as bacc
    import concourse.tile as tile
    from concourse import mybir

    F32 = mybir.dt.float32
    FP16 = mybir.dt.float16
    ADD = mybir.AluOpType.add
    EXP = mybir.ActivationFunctionType.Exp
    COPY = mybir.ActivationFunctionType.Copy

    nc = bacc.Bacc()
    # layouts chosen for long per-partition contiguous runs (big DMA
    # descriptors) and few dma_start jobs (sequencer trigger cost)
    xt = nc.declare_dram_parameter("xt", [GPC, P, EC, N], FP16, isOutput=False)
    wqk = nc.declare_dram_parameter(
        "wqk", [2, 2, P, 4, EC, P], FP16, isOutput=False
    )  # [which, ft-half, p, ft-lo, c, 128]
    wv = nc.declare_dram_parameter("wv", [P, EC, E], FP16, isOutput=False)
    wo = nc.declare_dram_parameter("wo", [P, FB, EC, 512], FP16, isOutput=False)
    # the problem spec pins bqkv/bout to zeros, so no bias paths exist:
    # q/k and out-proj evacuations are plain copies; the tail out-proj
    # evacuations run on the (then idle) scalar engine as table-free
    # Copy activations so DVE only carries the normalize stream.
    selb = nc.declare_dram_parameter("selb", [DH, DH * DH], FP16, isOutput=False)
    selc = nc.declare_dram_parameter("selc", [16, 8 * P], FP16, isOutput=False)
    out = nc.declare_dram_parameter("out", [TOK, E], FP16, isOutput=True)

    with nc.allow_low_precision(reason="fp16 softmax-denominator reciprocal"), \
        tile.TileContext(nc) as tc, tc.tile_pool(name="const", bufs=1) as const, \
        tc.tile_pool(name="xtp", bufs=2) as xtp, \
        tc.tile_pool(name="wqkp", bufs=4) as wqkp, \
        tc.tile_pool(name="qkp", bufs=10) as qkp, \
        tc.tile_pool(name="vfp", bufs=4) as vfp, \
        tc.tile_pool(name="expp", bufs=4) as expp, \
        tc.tile_pool(name="osbp", bufs=18) as osbp, \
        tc.tile_pool(name="denp", bufs=2) as denp, \
        tc.tile_pool(name="otp", bufs=16) as otp, \
        tc.tile_pool(name="outp", bufs=4) as outp, \
        tc.tile_pool(name="psmm", bufs=2, space="PSUM") as psmm, \
        tc.tile_pool(name="pssc", bufs=2, space="PSUM") as pssc, \
        tc.tile_pool(name="psop", bufs=2, space="PSUM") as psop:

        # ---- resident weights / constants ------------------------------
        # Few big DMA jobs on the sync HWDGE queue; jobs complete in issue
        # order at ~370GB/s aggregate, so order = first-need order.  bqk
        # goes second: the W1 bias-adds are what recycle the qk PSUM tiles.
        wqk_half = {}  # (which, fthalf) -> [128, 4, EC, 128]

        def load_wqk(which, fh):
            t = wqkp.tile(
                [P, 4, EC, P], FP16, tag="wqk", name=f"w{which}_{fh}"
            )
            nc.sync.dma_start(out=t, in_=wqk[which, fh])
            wqk_half[(which, fh)] = t

        # wqk(0,0) is split into per-ft tiles and xt0 into half tiles so
        # the first q chains start as soon as their own slices land
        # (deps are tile-granular: a split job into one tile would still
        # gate every reader on ALL the sub-jobs).
        w00f = []
        for fl in range(4):
            t = wqkp.tile([P, EC, P], FP16, tag="w00f", name=f"w00f{fl}")
            w00f.append(t)
        xt0h = [
            xtp.tile([P, 4, N], FP16, tag="xt0h", name=f"xt0{h}")
            for h in range(2)
        ]
        # Jobs within one HWDGE queue interleave across the DMA engines, so
        # everything outstanding completes roughly together: the only lever
        # is keeping each queue's early byte-count small.  The tiny critical
        # prefix (xt0 halves + bqk, ~1MB) goes on the otherwise-idle scalar
        # queue; the weight stream and the late consts stay on sync in
        # need order.
        nc.sync.dma_start(out=w00f[0], in_=wqk[0, 0, :, 0])
        nc.sync.dma_start(out=xt0h[0], in_=xt[0, :, 0:4])
        nc.sync.dma_start(out=w00f[1], in_=wqk[0, 0, :, 1])
        nc.sync.dma_start(out=xt0h[1], in_=xt[0, :, 4:8])
        nc.sync.dma_start(out=w00f[2], in_=wqk[0, 0, :, 2])
        nc.sync.dma_start(out=w00f[3], in_=wqk[0, 0, :, 3])
        load_wqk(0, 1)
        load_wqk(1, 0)
        load_wqk(1, 1)
        xt1_sb = xtp.tile([P, EC, N], FP16, tag="xt", name="xt1")
        nc.sync.dma_start(out=xt1_sb, in_=xt[1])
        wv_sb = const.tile([P, EC, E], FP16)
        nc.sync.dma_start(out=wv_sb, in_=wv[:])
        wo_sb = const.tile([P, FB, EC, 512], FP16)
        nc.sync.dma_start(out=wo_sb, in_=wo[:])

        def xt_c(g, c):
            # [P, N] access to contraction chunk c of group g's tokens
            if g == 0:
                return xt0h[c // 4][:, c % 4]
            return xt1_sb[:, c]

        # selector for the reciprocal-broadcast matmuls.  Slice (4*pr+c) of
        # sel_big is a [64,128] lhsT whose only nonzeros route rec4 row
        # 4*(2pr)+c (even head, chunk c) to output partitions 0:64 and row
        # 4*(2pr+1)+c to partitions 64:128 (host-built constant; first
        # needed in W3).
        sel_big = const.tile([DH, DH * DH], FP16)
        nc.sync.dma_start(out=sel_big, in_=selb[:])
        sel_c = const.tile([16, 8 * P], FP16)
        nc.sync.dma_start(out=sel_c, in_=selc[:])

        def wqk_tile(which, ft):
            if which == 0 and ft < 4:
                return w00f[ft]
            return wqk_half[(which, ft // 4)][:, ft % 4]

        # ---- state -----------------------------------------------------
        qts = {0: {}, 1: {}}
        kts = {0: {}, 1: {}}
        vfl = {0: [], 1: []}  # per tt: [128, 16, 128] (head blk = v|ones)
        exs = {}
        osbs = {0: {}, 1: {}}
        den4 = {}  # per g: [48,128] (heads 0-11), partition 4*h + chunk
        den4b = {}  # per g: [16,128] (heads 12-15) at base partition 0,
        rec4 = {}  # so pairs 6-7 can invert right after the last PV lands
        rec4b = {}
        ots = {0: {}, 1: {}}
        pv_pending = []  # deferred PV emission (software pipeline lag)

        def emit_qk_chain(g, which, ft, alt=True):
            # alt: alternate psmm/psop for 4-deep chain pipelining (only when
            # the attention PV isn't competing for psop)
            use_op = alt and (ft % 2 == 1)
            ps = (psop if use_op else psmm).tile(
                [P, N], F32, tag="po" if use_op else "mm"
            )
            wt = wqk_tile(which, ft)
            for c in range(EC):
                nc.tensor.matmul(
                    ps,
                    lhsT=wt[:, c, :],
                    rhs=xt_c(g, c),
                    start=(c == 0),
                    stop=(c == EC - 1),
                )
            t = qkp.tile([P, N], FP16, tag="qt" if which == 0 else "kt")
            nc.vector.tensor_copy(out=t, in_=ps)
            (qts if which == 0 else kts)[g][ft] = t

        def alloc_v_tiles(g):
            for t2 in range(NB):
                # [128 k-tok, 16 heads, 128]: head block = [v_h | ones]
                # (even h) or [ones | v_h] (odd h) so PV yields o rows on
                # one partition half and denominator rows on the other
                vt = vfp.tile([P, H, P], FP16, tag="vf")
                nc.vector.memset(vt[:, 0:H:2, DH:P], 1.0)
                nc.vector.memset(vt[:, 1:H:2, 0:DH], 1.0)
                vfl[g].append(vt)

        def emit_v_unit(g, fb, tt):
            if fb == 0 and tt == 0:
                alloc_v_tiles(g)
            use_op = tt % 2 == 1
            ps = (psop if use_op else psmm).tile(
                [P, 512], F32, tag="po" if use_op else "mm"
            )
            for c in range(EC):
                nc.tensor.matmul(
                    ps,
                    lhsT=xt_c(g, c)[:, tt * P : (tt + 1) * P],
                    rhs=wv_sb[:, c, fb * 512 : (fb + 1) * 512],
                    start=(c == 0),
                    stop=(c == EC - 1),
                )
            # scatter the 8 heads' v into the interleaved layout with two
            # strided copies (even heads -> block cols 0:64, odd -> 64:128)
            vt = vfl[g][tt]
            psv = ps.rearrange("p (j o) -> p j o", j=NB, o=P)
            h0 = fb * EC
            nc.vector.tensor_copy(
                out=vt[:, h0 : h0 + EC : 2, 0:DH], in_=psv[:, :, 0:DH]
            )
            nc.vector.tensor_copy(
                out=vt[:, h0 + 1 : h0 + EC : 2, DH:P], in_=psv[:, :, DH:P]
            )

        def emit_scores_half(g, pr, half):
            he, ho = 2 * pr, 2 * pr + 1
            if half == 0:
                for h in (he, ho):
                    exs[(g, h)] = expp.tile(
                        [P, NB, N], FP16, tag="exp", name=f"ex{g}_{h}"
                    )
            sce = pssc.tile([P, 2, N], F32, tag="sc")
            sco = pssc.tile([P, 2, N], F32, tag="sc")
            # alternate PE row groups (0-63 / 64-127) so the two heads'
            # K=64 matmuls stream concurrently on the array
            for cc in range(2):
                c = 2 * half + cc
                nc.tensor.matmul(
                    sce[:, cc],
                    lhsT=kts[g][pr][0:DH, c * P : (c + 1) * P],
                    rhs=qts[g][pr][0:DH, :],
                    start=True,
                    stop=True,
                )
                nc.tensor.matmul(
                    sco[:, cc],
                    lhsT=kts[g][pr][DH:P, c * P : (c + 1) * P],
                    rhs=qts[g][pr][DH:P, :],
                    start=True,
                    stop=True,
                )
            nc.scalar.activation(
                out=exs[(g, he)][:, 2 * half : 2 * half + 2], in_=sce, func=EXP
            )
            nc.scalar.activation(
                out=exs[(g, ho)][:, 2 * half : 2 * half + 2], in_=sco, func=EXP
            )

        def emit_pv(g, pr):
            if g not in den4:
                den4[g] = denp.tile([48, P], FP16, tag="den", name=f"den{g}")
                den4b[g] = denp.tile([16, P], FP16, tag="den", name=f"denb{g}")
            for h in (2 * pr, 2 * pr + 1):
                po = psop.tile([P, N], F32, tag="po")
                ex = exs[(g, h)]
                for c in range(NB):
                    nc.tensor.matmul(
                        po,
                        lhsT=vfl[g][c][:, h, :],
                        rhs=ex[:, c, :],
                        start=(c == 0),
                        stop=(c == NB - 1),
                    )
                osb = osbp.tile([P, N], FP16, tag="osb")
                nc.vector.tensor_copy(out=osb, in_=po)
                osbs[g][h] = osb
                # gather this head's replicated denominator row transposed
                # into den4: src [1,512] -> dst 4 contiguous partitions x
                # 128 (chunk-major), partition index 4*h + chunk
                dr = DH if h % 2 == 0 else 0
                src = osb[dr : dr + 1, :]
                if h < 12:
                    dst = den4[g][4 * h : 4 * h + 4, :]
                else:
                    dst = den4b[g][4 * (h - 12) : 4 * (h - 12) + 4, :]
                nc.gpsimd.dma_start(out=dst, in_=src)

        def flush_pv():
            while pv_pending:
                g, pr = pv_pending.pop(0)
                emit_pv(g, pr)

        def queue_pv(g, pr):
            pv_pending.append((g, pr))

        def emit_recip4(g, part):
            # part 0 -> pairs 0-3 (rows 0:32), 1 -> pairs 4-5 (32:48),
            # 2 -> pairs 6-7 (den4b)
            if g not in rec4:
                rec4[g] = denp.tile([48, P], FP16, tag="rec", name=f"rec{g}")
                rec4b[g] = denp.tile([16, P], FP16, tag="rec", name=f"recb{g}")
            if part == 2:
                nc.vector.reciprocal(out=rec4b[g], in_=den4b[g])
            else:
                s = slice(32 * part, 32 * part + (32 if part == 0 else 16))
                nc.vector.reciprocal(out=rec4[g][s, :], in_=den4[g][s, :])

        def emit_bcast_norm_pair(g, pr):
            # broadcast the pair's reciprocal rows across the partition
            # halves with 4 K=2 selector matmuls (chunk c -> cols c*128),
            # then normalize straight out of PSUM
            bc = psop.tile([P, N], F32, tag="po")
            # operand slices share a legal base partition (0 or 32): pairs
            # 0-3 in rec4[0:32], 4-5 in rec4[32:48], 6-7 in rec4b[0:16]
            if pr < 6:
                s = slice(32 * (pr // 4), 32 * (pr // 4) + (32 if pr < 4 else 16))
                r4 = rec4[g][s, :]
                sel_of = lambda c: sel_big[s, (4 * pr + c) * P : (4 * pr + c + 1) * P]
            else:
                r4 = rec4b[g]
                sel_of = lambda c: sel_c[:, (4 * (pr - 6) + c) * P : (4 * (pr - 6) + c + 1) * P]
            for c in range(NB):
                nc.tensor.matmul(
                    bc[:, c * P : (c + 1) * P],
                    lhsT=sel_of(c),
                    rhs=r4,
                    start=True,
                    stop=True,
                )
            he, ho = 2 * pr, 2 * pr + 1
            ot = otp.tile([P, N], FP16, tag="ot")
            nc.vector.tensor_mul(
                out=ot[0:DH, :], in0=osbs[g][he][0:DH, :], in1=bc[0:DH, :]
            )
            nc.vector.tensor_mul(
                out=ot[DH:P, :], in0=osbs[g][ho][DH:P, :], in1=bc[DH:P, :]
            )
            ots[g][pr] = ot

        def emit_outproj_unit(g, u, alt=False, evac="v"):
            fb, tt = u // NB, u % NB
            use_op = alt and (u % 2 == 1)
            ps = (psop if use_op else psmm).tile(
                [P, 512], F32, tag="po" if use_op else "mm"
            )
            for dc in range(EC):
                nc.tensor.matmul(
                    ps,
                    lhsT=ots[g][dc][:, tt * P : (tt + 1) * P],
                    rhs=wo_sb[:, fb, dc, :],
                    start=(dc == 0),
                    stop=(dc == EC - 1),
                )
            ob = outp.tile([P, 512], FP16, tag="ob")
            if evac == "s":
                nc.scalar.activation(out=ob, in_=ps, func=COPY)
            else:
                nc.vector.tensor_copy(out=ob, in_=ps)
            nc.sync.dma_start(
                out=out[
                    g * N + tt * P : g * N + (tt + 1) * P, fb * 512 : (fb + 1) * 512
                ],
                in_=ob,
            )

        # ---- program order ---------------------------------------------
        # W1: qkv(g0).  The first q chain paces with the xt DMA stream.
        for ft in range(EC):
            emit_qk_chain(0, 0, ft)
        for ft in range(EC):
            emit_qk_chain(0, 1, ft)
        for fb in range(FB):
            for tt in range(NB):
                emit_v_unit(0, fb, tt)

        # W2: attn(g0) with qk(g1) as PE filler while ACT runs the exps.
        # PV for pair pr is emitted one pair late (scores->exp latency) and
        # AFTER both qk chains, so both bias-adds precede the PV CASTs in
        # DVE order and the next pr's chain never stalls on its PSUM slot.
        # (the ft7 chains are deferred to W4's ACT-bound pr5/pr6 slots)
        for pr in range(EC):
            emit_scores_half(0, pr, 0)
            emit_scores_half(0, pr, 1)
            if pr < 7:
                emit_qk_chain(1, 0, pr, alt=False)
                emit_qk_chain(1, 1, pr, alt=False)
            flush_pv()
            queue_pv(0, pr)
        flush_pv()

        # W3: v(g1); meanwhile DVE inverts the g0 denominators ([64,128]
        # transposed layout -> ~1us) and the g0 bcast+normalize pairs run
        # interleaved with the fb=1 v-units, so all g0 ot tiles are ready
        # well before W4's outproj filler needs them.  The reciprocal
        # halves are emitted BEFORE the v-tile memsets/scatters they'd
        # otherwise queue behind on the in-order DVE.
        emit_recip4(0, 0)
        for tt in range(NB):
            emit_v_unit(1, 0, tt)
            if tt == 0:
                emit_recip4(0, 1)
                emit_recip4(0, 2)
        for tt in range(NB):
            emit_v_unit(1, 1, tt)
            emit_bcast_norm_pair(0, 2 * tt)
            emit_bcast_norm_pair(0, 2 * tt + 1)

        # W4: attn(g1) with outproj(g0) as PE filler for prs 0-5.  The g1
        # denominator pipeline starts mid-W4: pairs 0-3 invert after PV pr3
        # landed, and their bcast+normalize runs as pr6/pr7 filler, so only
        # pairs 4-7 remain for the tail.
        for pr in range(EC):
            emit_scores_half(1, pr, 0)
            emit_scores_half(1, pr, 1)
            if pr < 6:
                emit_outproj_unit(0, pr)
            if pr == 5:
                emit_qk_chain(1, 0, 7, alt=False)
            elif pr == 6:
                emit_qk_chain(1, 1, 7, alt=False)
            flush_pv()
            queue_pv(1, pr)
            if pr == 5:
                emit_recip4(1, 0)
            elif pr == 6:
                emit_recip4(1, 1)
                emit_bcast_norm_pair(1, 0)
                emit_bcast_norm_pair(1, 1)
            elif pr == 7:
                emit_bcast_norm_pair(1, 2)
                emit_bcast_norm_pair(1, 3)
                emit_bcast_norm_pair(1, 4)
                emit_bcast_norm_pair(1, 5)
        flush_pv()

        # W5: invert the last g1 denominators (pairs 6-7, base-0 tile so
        # the reciprocal can start right after the last PV), bcast +
        # normalize interleaved with the two reserved outproj(g0) units,
        # then out-proj g1 (alternating PSUM pools) and store.
        emit_recip4(1, 2)
        emit_outproj_unit(0, 6, evac="s")
        emit_bcast_norm_pair(1, 6)
        emit_outproj_unit(0, 7, evac="s")
        emit_bcast_norm_pair(1, 7)
        for u in range(EC):
            emit_outproj_unit(1, u, alt=True, evac="s")
    nc.finalize()
    return nc


def _get_nc():
    if "nc" not in _CACHE:
        _CACHE["nc"] = _build_nc()
    return _CACHE["nc"]


def _make_in_maps(x, Wqkv, bqkv, Wout, bout):
    """Host-side sharding: permute tokens to group-major, pre-transpose x,
    pack weights into DMA-friendly resident layouts."""
    x = np.asarray(x, dtype=np.float32)
    Wqkv = np.asarray(Wqkv, dtype=np.float32)
    bqkv = np.asarray(bqkv, dtype=np.float32)
    Wout = np.asarray(Wout, dtype=np.float32)
    bout = np.asarray(bout, dtype=np.float32)

    # group-major token order: x_perm[b, g*N + i] = x[b, i*ST + g]
    x_perm = x.reshape(B, N, ST, E).transpose(0, 2, 1, 3)  # [B, ST, N, E]

    # wqk[which][fh][p][fl][c][j] = W[c*128+p, (fh*4+fl)*128+j]  (q scaled)
    def tile_qk(w):
        return w.reshape(EC, P, 2, 4, P).transpose(2, 1, 3, 0, 4)

    wqk = np.ascontiguousarray(
        np.stack(
            [tile_qk(Wqkv[:, 0:E] * SCALE), tile_qk(Wqkv[:, E : 2 * E])], axis=0
        ).astype(np.float16)
    )
    # wv[p][c][f] = Wv[c*128+p, f]
    wv = np.ascontiguousarray(
        Wqkv[:, 2 * E : 3 * E].reshape(EC, P, E).transpose(1, 0, 2).astype(np.float16)
    )
    # wo[p][fb][dc][j] = Wout[dc*128+p, fb*512+j]
    wo = np.ascontiguousarray(
        Wout.reshape(EC, P, FB, 512).transpose(1, 2, 0, 3).astype(np.float16)
    )
    # the spec pins bqkv/bout to zeros; assert and drop all bias paths
    assert not np.any(bqkv) and not np.any(bout), "kernel assumes zero biases"
    # reciprocal-broadcast selectors: slice m=4*pr+c routes den4 row
    # 4*(2pr)+c to partition half 0:64 and row 4*(2pr+1)+c to 64:128;
    # pairs 6-7 live in their own base-0 tile with selector selc
    selb = np.zeros((DH, DH * DH), dtype=np.float16)
    for m in range(24):
        pr, c = divmod(m, 4)
        selb[4 * (2 * pr) + c, m * P : m * P + DH] = 1.0
        selb[4 * (2 * pr + 1) + c, m * P + DH : (m + 1) * P] = 1.0
    selc = np.zeros((16, 8 * P), dtype=np.float16)
    for m in range(8):
        pr, c = divmod(m, 4)
        selc[4 * (2 * pr) + c, m * P : m * P + DH] = 1.0
        selc[4 * (2 * pr + 1) + c, m * P + DH : (m + 1) * P] = 1.0

    in_maps = []
    for core in range(NCORES):
        b = core // (NCORES // B)
        g0 = GPC * (core % (NCORES // B))
        xc = x_perm[b, g0 : g0 + GPC].reshape(TOK, E)  # [1024, E]
        # xt[g][p][c][t] = x[g*N + t, c*128 + p]
        xct = np.ascontiguousarray(
            xc.T.reshape(EC, P, GPC, N).transpose(2, 1, 0, 3).astype(np.float16)
        )
        in_maps.append(
            {
                "xt": xct,
                "wqk": wqk,
                "wv": wv,
                "wo": wo,
                "selb": selb,
                "selc": selc,
            }
        )
    return in_maps


def kernel(x, Wqkv, bqkv, Wout, bout):
    from concourse.bass_utils import run_bass_kernel_spmd

    nc = _get_nc()
    in_maps = _make_in_maps(x, Wqkv, bqkv, Wout, bout)
    trace = bool(int(os.environ.get("KERNEL_TRACE", "0")))
    res = run_bass_kernel_spmd(
        nc, in_maps, core_ids=list(range(NCORES)), trace=trace
    )
    _CACHE["last_result"] = res

    # reassemble: core outputs are [1024 tok, E] fp16 in group-major order
    out = np.empty((B, S, E), dtype=np.float32)
    for b in range(B):
        per_b = [
            np.asarray(res.results[b * (NCORES // B) + j]["out"], dtype=np.float32)
            for j in range(NCORES // B)
        ]
        perm = np.concatenate(per_b, axis=0)  # [ST*N, E] group-major
        out[b] = perm.reshape(ST, N, E).transpose(1, 0, 2).reshape(S, E)
    return out


# revision 61
# speedup vs baseline: 1.0616x; 1.0024x over previous
"""Strided (residue-group) attention for Trainium2, SPMD across 8 NeuronCores.

Problem: x[B=2,S=4096,E=1024] -> qkv proj -> per-(batch,head,residue-group)
attention (stride 8 -> 8 groups of n=512 tokens) -> out proj.

Sharding: by (batch, residue-group).  B*stride = 16 group-instances; each of
the 8 cores owns 2 (batch,group) pairs = 1024 tokens and computes their FULL
output rows (it holds all 16 heads for its tokens).  The residue groups are
independent, so there are no cross-device collectives at all; the host
permutes tokens into group-major order on the way in and inverts on the way
out.

v3 design notes (vs the v2 baseline at 264us):
  - bqk loads right after the first weight block (v2 loaded it LAST, so the
    W1 bias-adds stalled 12us and the PE blocked on PSUM recycling).
  - The softmax-denominator path is restructured.  v2 gathered den rows to
    [16,512], ran one DVE reciprocal (3.3us - reciprocal cost scales with
    the FREE dim at ~8 cyc/elem), then broadcast each row across partitions
    with 8 serialized SWDGE DMAs (~1us each + slow data movement): ~12us of
    dead time per group during which the in-order PE blocked behind outproj
    chains.  v3 gathers the den rows TRANSPOSED into [64,128] (4 chunks of
    128 q-tokens per head; free dim 128 -> reciprocal ~1us), then broadcasts
    reciprocal rows into PSUM with tiny K=2 selector matmuls on the PE
    (sel[2,128] lhsT picks the head-half; 4 matmuls x 128 cols per pair,
    ~0.2us) and the normalize multiplies read the PSUM tile directly.
  - g0's bcast+normalize now completes inside W3 (interleaved with the g1
    v-units), so W4's outproj(g0) filler never stalls; g1's runs in the tail
    interleaved with 4 reserved outproj(g0) filler units.
  - v-tile ones-blocks are memset only for g0; the vfp ring (4 bufs, 4
    tiles/group) hands g1 the same buffers with the ones intact.
  - Everything else as v2: ScalarE runs ONLY softmax Exp; weights resident
    in SBUF in DMA-friendly layouts; v-proj bias folded into the out-proj
    bias host-side; score matmuls alternate PE row groups; PV emitted one
    pair behind its scores; fp16 activations, fp16 output (host upcasts).
"""

import os

import numpy as np

B, S, E = 2, 4096, 1024
H, ST = 16, 8
DH = E // H  # 64
N = S // ST  # 512 tokens per residue group
NCORES = 8
GPC = (B * ST) // NCORES  # 2 (batch,group) pairs per core
TOK = GPC * N  # 1024 tokens per core
P = 128
EC = E // P  # 8 contraction chunks of 128
NB = N // P  # 4 token chunks of 128 per group
FB = 2  # feature blocks of 512 in E
SCALE = 1.0 / float(np.sqrt(DH))

_CACHE: dict = {}


def _build_nc():
    import concourse.bass as bass
    import concourse.bacc as bacc
    import concourse.tile as tile
    from concourse import mybir

    F32 = mybir.dt.float32
    FP16 = mybir.dt.float16
    ADD = mybir.AluOpType.add
    EXP = mybir.ActivationFunctionType.Exp
    COPY = mybir.ActivationFunctionType.Copy

    nc = bacc.Bacc()
    # layouts chosen for long per-partition contiguous runs (big DMA
    # descriptors) and few dma_start jobs (sequencer trigger cost)
    xt = nc.declare_dram_parameter("xt", [GPC, P, EC, N], FP16, isOutput=False)
    wqk = nc.declare_dram_parameter(
        "wqk", [2, 2, P, 4, EC, P], FP16, isOutput=False
    )  # [which, ft-half, p, ft-lo, c, 128]
    wv = nc.declare_dram_parameter("wv", [P, EC, E], FP16, isOutput=False)
    wo = nc.declare_dram_parameter("wo", [P, FB, EC, 512], FP16, isOutput=False)
    # the problem spec pins bqkv/bout to zeros, so no bias paths exist:
    # q/k and out-proj evacuations are plain copies; the tail out-proj
    # evacuations run on the (then idle) scalar engine as table-free
    # Copy activations so DVE only carries the normalize stream.
    selb = nc.declare_dram_parameter("selb", [DH, DH * DH], FP16, isOutput=False)
    selc = nc.declare_dram_parameter("selc", [16, 8 * P], FP16, isOutput=False)
    out = nc.declare_dram_parameter("out", [TOK, E], FP16, isOutput=True)

    with nc.allow_low_precision(reason="fp16 softmax-denominator reciprocal"), \
        tile.TileContext(nc) as tc, tc.tile_pool(name="const", bufs=1) as const, \
        tc.tile_pool(name="xtp", bufs=2) as xtp, \
        tc.tile_pool(name="wqkp", bufs=4) as wqkp, \
        tc.tile_pool(name="qkp", bufs=10) as qkp, \
        tc.tile_pool(name="vfp", bufs=4) as vfp, \
        tc.tile_pool(name="expp", bufs=4) as expp, \
        tc.tile_pool(name="osbp", bufs=18) as osbp, \
        tc.tile_pool(name="denp", bufs=2) as denp, \
        tc.tile_pool(name="otp", bufs=16) as otp, \
        tc.tile_pool(name="outp", bufs=4) as outp, \
        tc.tile_pool(name="psmm", bufs=2, space="PSUM") as psmm, \
        tc.tile_pool(name="pssc", bufs=2, space="PSUM") as pssc, \
        tc.tile_pool(name="psop", bufs=2, space="PSUM") as psop:

        # ---- resident weights / constants ------------------------------
        # Few big DMA jobs on the sync HWDGE queue; jobs complete in issue
        # order at ~370GB/s aggregate, so order = first-need order.  bqk
        # goes second: the W1 bias-adds are what recycle the qk PSUM tiles.
        wqk_half = {}  # (which, fthalf) -> [128, 4, EC, 128]

        def load_wqk(which, fh):
            t = wqkp.tile(
                [P, 4, EC, P], FP16, tag="wqk", name=f"w{which}_{fh}"
            )
            nc.sync.dma_start(out=t, in_=wqk[which, fh])
            wqk_half[(which, fh)] = t

        # wqk(0,0) is split into per-ft tiles and xt0 into half tiles so
        # the first q chains start as soon as their own slices land
        # (deps are tile-granular: a split job into one tile would still
        # gate every reader on ALL the sub-jobs).
        w00f = []
        for fl in range(4):
            t = wqkp.tile([P, EC, P], FP16, tag="w00f", name=f"w00f{fl}")
            w00f.append(t)
        xt0h = [
            xtp.tile([P, 4, N], FP16, tag="xt0h", name=f"xt0{h}")
            for h in range(2)
        ]
        # Jobs within one HWDGE queue interleave across the DMA engines, so
        # everything outstanding completes roughly together: the only lever
        # is keeping each queue's early byte-count small.  The tiny critical
        # prefix (xt0 halves + bqk, ~1MB) goes on the otherwise-idle scalar
        # queue; the weight stream and the late consts stay on sync in
        # need order.
        nc.sync.dma_start(out=w00f[0], in_=wqk[0, 0, :, 0])
        nc.sync.dma_start(out=xt0h[0], in_=xt[0, :, 0:4])
        nc.sync.dma_start(out=w00f[1], in_=wqk[0, 0, :, 1])
        nc.sync.dma_start(out=xt0h[1], in_=xt[0, :, 4:8])
        nc.sync.dma_start(out=w00f[2], in_=wqk[0, 0, :, 2])
        nc.sync.dma_start(out=w00f[3], in_=wqk[0, 0, :, 3])
        load_wqk(0, 1)
        load_wqk(1, 0)
        load_wqk(1, 1)
        xt1_sb = xtp.tile([P, EC, N], FP16, tag="xt", name="xt1")
        nc.sync.dma_start(out=xt1_sb, in_=xt[1])
        wv_sb = const.tile([P, EC, E], FP16)
        nc.sync.dma_start(out=wv_sb, in_=wv[:])
        wo_sb = const.tile([P, FB, EC, 512], FP16)
        nc.sync.dma_start(out=wo_sb, in_=wo[:])

        def xt_c(g, c):
            # [P, N] access to contraction chunk c of group g's tokens
            if g == 0:
                return xt0h[c // 4][:, c % 4]
            return xt1_sb[:, c]

        # selector for the reciprocal-broadcast matmuls.  Slice (4*pr+c) of
        # sel_big is a [64,128] lhsT whose only nonzeros route rec4 row
        # 4*(2pr)+c (even head, chunk c) to output partitions 0:64 and row
        # 4*(2pr+1)+c to partitions 64:128 (host-built constant; first
        # needed in W3).
        sel_big = const.tile([DH, DH * DH], FP16)
        nc.sync.dma_start(out=sel_big, in_=selb[:])
        sel_c = const.tile([16, 8 * P], FP16)
        nc.sync.dma_start(out=sel_c, in_=selc[:])

        def wqk_tile(which, ft):
            if which == 0 and ft < 4:
                return w00f[ft]
            return wqk_half[(which, ft // 4)][:, ft % 4]

        # ---- state -----------------------------------------------------
        qts = {0: {}, 1: {}}
        kts = {0: {}, 1: {}}
        vfl = {0: [], 1: []}  # per tt: [128, 16, 128] (head blk = v|ones)
        exs = {}
        osbs = {0: {}, 1: {}}
        den4 = {}  # per g: [48,128] (heads 0-11), partition 4*h + chunk
        den4b = {}  # per g: [16,128] (heads 12-15) at base partition 0,
        rec4 = {}  # so pairs 6-7 can invert right after the last PV lands
        rec4b = {}
        ots = {0: {}, 1: {}}
        pv_pending = []  # deferred PV emission (software pipeline lag)

        def emit_qk_chain(g, which, ft, alt=True):
            # alt: alternate psmm/psop for 4-deep chain pipelining (only when
            # the attention PV isn't competing for psop)
            use_op = alt and (ft % 2 == 1)
            ps = (psop if use_op else psmm).tile(
                [P, N], F32, tag="po" if use_op else "mm"
            )
            wt = wqk_tile(which, ft)
            for c in range(EC):
                nc.tensor.matmul(
                    ps,
                    lhsT=wt[:, c, :],
                    rhs=xt_c(g, c),
                    start=(c == 0),
                    stop=(c == EC - 1),
                )
            t = qkp.tile([P, N], FP16, tag="qt" if which == 0 else "kt")
            nc.vector.tensor_copy(out=t, in_=ps)
            (qts if which == 0 else kts)[g][ft] = t

        def alloc_v_tiles(g):
            for t2 in range(NB):
                # [128 k-tok, 16 heads, 128]: head block = [v_h | ones]
                # (even h) or [ones | v_h] (odd h) so PV yields o rows on
                # one partition half and denominator rows on the other
                vt = vfp.tile([P, H, P], FP16, tag="vf")
                nc.vector.memset(vt[:, 0:H:2, DH:P], 1.0)
                nc.vector.memset(vt[:, 1:H:2, 0:DH], 1.0)
                vfl[g].append(vt)

        def emit_v_unit(g, fb, tt):
            if fb == 0 and tt == 0:
                alloc_v_tiles(g)
            use_op = tt % 2 == 1
            ps = (psop if use_op else psmm).tile(
                [P, 512], F32, tag="po" if use_op else "mm"
            )
            for c in range(EC):
                nc.tensor.matmul(
                    ps,
                    lhsT=xt_c(g, c)[:, tt * P : (tt + 1) * P],
                    rhs=wv_sb[:, c, fb * 512 : (fb + 1) * 512],
                    start=(c == 0),
                    stop=(c == EC - 1),
                )
            # scatter the 8 heads' v into the interleaved layout with two
            # strided copies (even heads -> block cols 0:64, odd -> 64:128)
            vt = vfl[g][tt]
            psv = ps.rearrange("p (j o) -> p j o", j=NB, o=P)
            h0 = fb * EC
            nc.vector.tensor_copy(
                out=vt[:, h0 : h0 + EC : 2, 0:DH], in_=psv[:, :, 0:DH]
            )
            nc.vector.tensor_copy(
                out=vt[:, h0 + 1 : h0 + EC : 2, DH:P], in_=psv[:, :, DH:P]
            )

        def emit_scores_half(g, pr, half):
            he, ho = 2 * pr, 2 * pr + 1
            if half == 0:
                for h in (he, ho):
                    exs[(g, h)] = expp.tile(
                        [P, NB, N], FP16, tag="exp", name=f"ex{g}_{h}"
                    )
            sce = pssc.tile([P, 2, N], F32, tag="sc")
            sco = pssc.tile([P, 2, N], F32, tag="sc")
            # alternate PE row groups (0-63 / 64-127) so the two heads'
            # K=64 matmuls stream concurrently on the array
            for cc in range(2):
                c = 2 * half + cc
                nc.tensor.matmul(
                    sce[:, cc],
                    lhsT=kts[g][pr][0:DH, c * P : (c + 1) * P],
                    rhs=qts[g][pr][0:DH, :],
                    start=True,
                    stop=True,
                )
                nc.tensor.matmul(
                    sco[:, cc],
                    lhsT=kts[g][pr][DH:P, c * P : (c + 1) * P],
                    rhs=qts[g][pr][DH:P, :],
                    start=True,
                    stop=True,
                )
            nc.scalar.activation(
                out=exs[(g, he)][:, 2 * half : 2 * half + 2], in_=sce, func=EXP
            )
            nc.scalar.activation(
                out=exs[(g, ho)][:, 2 * half : 2 * half + 2], in_=sco, func=EXP
            )

        def emit_pv(g, pr):
            if g not in den4:
                den4[g] = denp.tile([48, P], FP16, tag="den", name=f"den{g}")
                den4b[g] = denp.tile([16, P], FP16, tag="den", name=f"denb{g}")
            for h in (2 * pr, 2 * pr + 1):
                po = psop.tile([P, N], F32, tag="po")
                ex = exs[(g, h)]
                for c in range(NB):
                    nc.tensor.matmul(
                        po,
                        lhsT=vfl[g][c][:, h, :],
                        rhs=ex[:, c, :],
                        start=(c == 0),
                        stop=(c == NB - 1),
                    )
                osb = osbp.tile([P, N], FP16, tag="osb")
                nc.vector.tensor_copy(out=osb, in_=po)
                osbs[g][h] = osb
                # gather this head's replicated denominator row transposed
                # into den4: src [1,512] -> dst 4 contiguous partitions x
                # 128 (chunk-major), partition index 4*h + chunk
                dr = DH if h % 2 == 0 else 0
                src = osb[dr : dr + 1, :]
                if h < 12:
                    dst = den4[g][4 * h : 4 * h + 4, :]
                else:
                    dst = den4b[g][4 * (h - 12) : 4 * (h - 12) + 4, :]
                nc.gpsimd.dma_start(out=dst, in_=src)

        def flush_pv():
            while pv_pending:
                g, pr = pv_pending.pop(0)
                emit_pv(g, pr)

        def queue_pv(g, pr):
            pv_pending.append((g, pr))

        def emit_recip4(g, part):
            # part 0 -> pairs 0-3 (rows 0:32), 1 -> pairs 4-5 (32:48),
            # 2 -> pairs 6-7 (den4b)
            if g not in rec4:
                rec4[g] = denp.tile([48, P], FP16, tag="rec", name=f"rec{g}")
                rec4b[g] = denp.tile([16, P], FP16, tag="rec", name=f"recb{g}")
            if part == 2:
                nc.vector.reciprocal(out=rec4b[g], in_=den4b[g])
            else:
                s = slice(32 * part, 32 * part + (32 if part == 0 else 16))
                nc.vector.reciprocal(out=rec4[g][s, :], in_=den4[g][s, :])

        def emit_bcast_norm_pair(g, pr):
            # broadcast the pair's reciprocal rows across the partition
            # halves with 4 K=2 selector matmuls (chunk c -> cols c*128),
            # then normalize straight out of PSUM
            bc = psop.tile([P, N], F32, tag="po")
            # operand slices share a legal base partition (0 or 32): pairs
            # 0-3 in rec4[0:32], 4-5 in rec4[32:48], 6-7 in rec4b[0:16]
            if pr < 6:
                s = slice(32 * (pr // 4), 32 * (pr // 4) + (32 if pr < 4 else 16))
                r4 = rec4[g][s, :]
                sel_of = lambda c: sel_big[s, (4 * pr + c) * P : (4 * pr + c + 1) * P]
            else:
                r4 = rec4b[g]
                sel_of = lambda c: sel_c[:, (4 * (pr - 6) + c) * P : (4 * (pr - 6) + c + 1) * P]
            for c in range(NB):
                nc.tensor.matmul(
                    bc[:, c * P : (c + 1) * P],
                    lhsT=sel_of(c),
                    rhs=r4,
                    start=True,
                    stop=True,
                )
            he, ho = 2 * pr, 2 * pr + 1
            ot = otp.tile([P, N], FP16, tag="ot")
            nc.vector.tensor_mul(
                out=ot[0:DH, :], in0=osbs[g][he][0:DH, :], in1=bc[0:DH, :]
            )
            nc.vector.tensor_mul(
                out=ot[DH:P, :], in0=osbs[g][ho][DH:P, :], in1=bc[DH:P, :]
            )
            ots[g][pr] = ot

        def emit_outproj_unit(g, u, alt=False, evac="v"):
            fb, tt = u // NB, u % NB
            use_op = alt and (u % 2 == 1)
            ps = (psop if use_op else psmm).tile(
                [P, 512], F32, tag="po" if use_op else "mm"
            )
            for dc in range(EC):
                nc.tensor.matmul(
                    ps,
                    lhsT=ots[g][dc][:, tt * P : (tt + 1) * P],
                    rhs=wo_sb[:, fb, dc, :],
                    start=(dc == 0),
                    stop=(dc == EC - 1),
                )
            ob = outp.tile([P, 512], FP16, tag="ob")
            if evac == "s":
                nc.scalar.activation(out=ob, in_=ps, func=COPY)
            else:
                nc.vector.tensor_copy(out=ob, in_=ps)
            nc.sync.dma_start(
                out=out[
                    g * N + tt * P : g * N + (tt + 1) * P, fb * 512 : (fb + 1) * 512
                ],
                in_=ob,
            )

        # ---- program order ---------------------------------------------
        # W1: qkv(g0).  The first q chain paces with the xt DMA stream.
        for ft in range(EC):
            emit_qk_chain(0, 0, ft)
        for ft in range(EC):
            emit_qk_chain(0, 1, ft)
        for fb in range(FB):
            for tt in range(NB):
                emit_v_unit(0, fb, tt)

        # W2: attn(g0) with qk(g1) as PE filler while ACT runs the exps.
        # PV for pair pr is emitted one pair late (scores->exp latency) and
        # AFTER both qk chains, so both bias-adds precede the PV CASTs in
        # DVE order and the next pr's chain never stalls on its PSUM slot.
        # (the ft7 chains are deferred to W4's ACT-bound pr5/pr6 slots)
        for pr in range(EC):
            emit_scores_half(0, pr, 0)
            emit_scores_half(0, pr, 1)
            if pr < 7:
                emit_qk_chain(1, 0, pr, alt=False)
                emit_qk_chain(1, 1, pr, alt=False)
            flush_pv()
            queue_pv(0, pr)
        flush_pv()

        # W3: v(g1); meanwhile DVE inverts the g0 denominators ([64,128]
        # transposed layout -> ~1us) and the g0 bcast+normalize pairs run
        # interleaved with the fb=1 v-units, so all g0 ot tiles are ready
        # well before W4's outproj filler needs them.  The reciprocal
        # halves are emitted BEFORE the v-tile memsets/scatters they'd
        # otherwise queue behind on the in-order DVE.
        emit_recip4(0, 0)
        for tt in range(NB):
            emit_v_unit(1, 0, tt)
            if tt == 0:
                emit_recip4(0, 1)
                emit_recip4(0, 2)
        for tt in range(NB):
            emit_v_unit(1, 1, tt)
            emit_bcast_norm_pair(0, 2 * tt)
            emit_bcast_norm_pair(0, 2 * tt + 1)

        # W4: attn(g1) with outproj(g0) as PE filler for prs 0-5.  The g1
        # denominator pipeline starts mid-W4: pairs 0-3 invert after PV pr3
        # landed, and their bcast+normalize runs as pr6/pr7 filler, so only
        # pairs 4-7 remain for the tail.
        for pr in range(EC):
            emit_scores_half(1, pr, 0)
            emit_scores_half(1, pr, 1)
            if pr < 6:
                emit_outproj_unit(0, pr)
            if pr == 5:
                emit_qk_chain(1, 0, 7, alt=False)
            elif pr == 6:
                emit_qk_chain(1, 1, 7, alt=False)
            flush_pv()
            queue_pv(1, pr)
            if pr == 5:
                emit_recip4(1, 0)
            elif pr == 6:
                emit_recip4(1, 1)
                emit_bcast_norm_pair(1, 0)
                emit_bcast_norm_pair(1, 1)
            elif pr == 7:
                emit_bcast_norm_pair(1, 2)
                emit_bcast_norm_pair(1, 3)
                emit_bcast_norm_pair(1, 4)
                emit_bcast_norm_pair(1, 5)
        flush_pv()

        # W5: invert the last g1 denominators (pairs 6-7, base-0 tile so
        # the reciprocal can start right after the last PV), bcast +
        # normalize interleaved with the two reserved outproj(g0) units,
        # then out-proj g1 (alternating PSUM pools) and store.
        emit_recip4(1, 2)
        emit_outproj_unit(0, 6, evac="s")
        emit_outproj_unit(0, 7, evac="s")
        emit_bcast_norm_pair(1, 6)
        emit_bcast_norm_pair(1, 7)
        for u in range(EC):
            emit_outproj_unit(1, u, alt=True, evac="s")
    nc.finalize()
    return nc


def _get_nc():
    if "nc" not in _CACHE:
        _CACHE["nc"] = _build_nc()
    return _CACHE["nc"]


def _make_in_maps(x, Wqkv, bqkv, Wout, bout):
    """Host-side sharding: permute tokens to group-major, pre-transpose x,
    pack weights into DMA-friendly resident layouts."""
    x = np.asarray(x, dtype=np.float32)
    Wqkv = np.asarray(Wqkv, dtype=np.float32)
    bqkv = np.asarray(bqkv, dtype=np.float32)
    Wout = np.asarray(Wout, dtype=np.float32)
    bout = np.asarray(bout, dtype=np.float32)

    # group-major token order: x_perm[b, g*N + i] = x[b, i*ST + g]
    x_perm = x.reshape(B, N, ST, E).transpose(0, 2, 1, 3)  # [B, ST, N, E]

    # wqk[which][fh][p][fl][c][j] = W[c*128+p, (fh*4+fl)*128+j]  (q scaled)
    def tile_qk(w):
        return w.reshape(EC, P, 2, 4, P).transpose(2, 1, 3, 0, 4)

    wqk = np.ascontiguousarray(
        np.stack(
            [tile_qk(Wqkv[:, 0:E] * SCALE), tile_qk(Wqkv[:, E : 2 * E])], axis=0
        ).astype(np.float16)
    )
    # wv[p][c][f] = Wv[c*128+p, f]
    wv = np.ascontiguousarray(
        Wqkv[:, 2 * E : 3 * E].reshape(EC, P, E).transpose(1, 0, 2).astype(np.float16)
    )
    # wo[p][fb][dc][j] = Wout[dc*128+p, fb*512+j]
    wo = np.ascontiguousarray(
        Wout.reshape(EC, P, FB, 512).transpose(1, 2, 0, 3).astype(np.float16)
    )
    # the spec pins bqkv/bout to zeros; assert and drop all bias paths
    assert not np.any(bqkv) and not np.any(bout), "kernel assumes zero biases"
    # reciprocal-broadcast selectors: slice m=4*pr+c routes den4 row
    # 4*(2pr)+c to partition half 0:64 and row 4*(2pr+1)+c to 64:128;
    # pairs 6-7 live in their own base-0 tile with selector selc
    selb = np.zeros((DH, DH * DH), dtype=np.float16)
    for m in range(24):
        pr, c = divmod(m, 4)
        selb[4 * (2 * pr) + c, m * P : m * P + DH] = 1.0
        selb[4 * (2 * pr + 1) + c, m * P + DH : (m + 1) * P] = 1.0
    selc = np.zeros((16, 8 * P), dtype=np.float16)
    for m in range(8):
        pr, c = divmod(m, 4)
        selc[4 * (2 * pr) + c, m * P : m * P + DH] = 1.0
        selc[4 * (2 * pr + 1) + c, m * P + DH : (m + 1) * P] = 1.0

    in_maps = []
    for core in range(NCORES):
        b = core // (NCORES // B)
        g0 = GPC * (core % (NCORES // B))
        xc = x_perm[b, g0 : g0 + GPC].reshape(TOK, E)  # [1024, E]
        # xt[g][p][c][t] = x[g*N + t, c*128 + p]
        xct = np.ascontiguousarray(
            xc.T.reshape(EC, P, GPC, N).transpose(2, 1, 0, 3).astype(np.float16)
        )
        in_maps.append(
            {
                "xt": xct,
                "wqk": wqk,
                "wv": wv,
                "wo": wo,
                "selb": selb,
                "selc": selc,
            }
        )
    return in_maps


def kernel(x, Wqkv, bqkv, Wout, bout):
    from concourse.bass_utils import run_bass_kernel_spmd

    nc = _get_nc()
    in_maps = _make_in_maps(x, Wqkv, bqkv, Wout, bout)
    trace = bool(int(os.environ.get("KERNEL_TRACE", "0")))
    res = run_bass_kernel_spmd(
        nc, in_maps, core_ids=list(range(NCORES)), trace=trace
    )
    _CACHE["last_result"] = res

    # reassemble: core outputs are [1024 tok, E] fp16 in group-major order
    out = np.empty((B, S, E), dtype=np.float32)
    for b in range(B):
        per_b = [
            np.asarray(res.results[b * (NCORES // B) + j]["out"], dtype=np.float32)
            for j in range(NCORES // B)
        ]
        perm = np.concatenate(per_b, axis=0)  # [ST*N, E] group-major
        out[b] = perm.reshape(ST, N, E).transpose(1, 0, 2).reshape(S, E)
    return out
